# revision 60
# baseline (speedup 1.0000x reference)
"""Trainium2 Bass kernel for BaseDependentAttentionLayer (GNN message passing).

v3 design (8 NeuronCores, SPMD), structured-slot layout:
  - Edges sharded by origin core. Within a core, each of 4 dest-chunks gets its
    OWN degree-sorted origin permutation: chunk-c slot (block b, partition p)
    holds one origin; tile t of block b holds the t-th chunk-c edge of each
    origin in the block (blocks padded to a uniform per-block degree).
  - Consequences: q is a per-partition broadcast from an SBUF table (no
    per-edge q gather); the scatter-reduction matmul uses a CONSTANT identity
    lhsT (no per-edge one-hot gather). Only ONE 256B gather per edge (k|v).
  - Chunk sizes are uneven ([8,16,17,15] slabs): a small chunk 0 shortens the
    prologue (table build) critical path; a smaller chunk 3 shortens the tail.
  - Per-chunk partials (vals|z) drain to DRAM rows [z f32 | vals bf16] 256B;
    chunk 3 drains straight into the SBUF accumulator, and the combine
    (3 per-node gathers + adds) plus the whole epilogue run interleaved with
    the chunk-3 pass per 14-block group.
  - Softmax pad slots hit a zero k|v row with ew=0 so they add exactly 1.0 to
    z; a host-computed npad tile subtracts them in the epilogue.
"""

import sys

sys.path.insert(0, "/opt/trn_rl_repo")

import numpy as np
import ml_dtypes

import concourse.bass as bass
import concourse.bacc as bacc
import concourse.mybir as mybir
from concourse.tile import TileContext
from concourse.bass_utils import run_bass_kernel_spmd

N = 100000
E = 1600000
D = 64
H = 4
HD = 16
NCORES = 8
NOWN = 12500            # nodes owned per core
NBLK = 98               # 128-node blocks per core (final order)
NB = NBLK * 128         # 12544 padded own nodes
SLAB = 1792
CH_SLABS = [8, 16, 17, 15]
CH_ROWS = [s * SLAB for s in CH_SLABS]
CH_R0 = np.concatenate([[0], np.cumsum(CH_ROWS)])   # len 5
CHN = 4
NT = int(CH_R0[-1])     # 100352
PZROW = NB              # zero row index within each partial table
SEG_TILES = 56          # max tiles per equal-degree segment
CWIN = 64               # compute-window tiles (merged elementwise ops)
EGRP = 14               # epilogue block-group size
LN_EPS = 1e-5

F32 = mybir.dt.float32
BF16 = mybir.dt.bfloat16
U32 = mybir.dt.uint32
I16 = mybir.dt.int16
BF16_NP = ml_dtypes.bfloat16

# engine knobs (tuned against the CoreSim cost model)
WS_ON_POOL = True
TREE1_ON_POOL = True
CTB_ON_POOL = False


def _wrap_idx(vals):
    """SWDGE index layout: [16, n/16] wrapped, replicated to 128 partitions."""
    assert len(vals) % 16 == 0
    w = vals.reshape(-1, 16).T.astype(np.int16)
    return np.tile(w, (8, 1))


def _slab_perm(nrows):
    """Column permutation making table-write DMAs contiguous (baseline)."""
    assert nrows % SLAB == 0
    j = np.arange(nrows)
    s, r = j // SLAB, j % SLAB
    a, p = r // 128, r % 128
    return s * SLAB + p * (SLAB // 128) + a


def _build_structure(origins, dests):
    """Common (cross-core max) per-chunk block-degree structure + per-core
    degree/sort data."""
    owner = origins // NOWN
    per_core = []
    for c in range(NCORES):
        m = owner == c
        o = (origins[m] - c * NOWN).astype(np.int32)
        d = dests[m].astype(np.int32)
        w_idx = np.nonzero(m)[0]
        per_core.append((o, d, w_idx))

    deg = np.zeros((NCORES, CHN, NB), np.int64)
    for ci, (o, d, w_idx) in enumerate(per_core):
        ch = np.searchsorted(CH_R0, d, side="right") - 1
        for c in range(CHN):
            deg[ci, c] = np.bincount(o[ch == c], minlength=NB)

    sdeg = -np.sort(-deg, axis=2)            # per-core sorted degree curves
    common = sdeg.max(axis=0)                # [CHN, NB] common slot degrees

    nblocks = []
    block_deg = []
    for c in range(CHN):
        if c < CHN - 1:
            nz = int((common[c] > 0).sum())
            nb = max((nz + 127) // 128, 1)
        else:
            nb = NBLK
        bd = common[c].reshape(NBLK, 128)[:nb, 0].copy()   # block max = first
        if c == CHN - 1:
            bd = np.maximum(bd, 1)           # every final block drains
        nblocks.append(nb)
        block_deg.append(bd.astype(np.int64))

    # segments: runs of equal block degree, capped at SEG_TILES tiles.
    # chunk-3 segments must not straddle EGRP boundaries (epilogue groups).
    segs = []          # (chunk, d, b0, nb, tile_off)
    t_off = 0
    for c in range(CHN):
        bd = block_deg[c]
        b = 0
        while b < nblocks[c]:
            dd = int(bd[b])
            e = b
            while e < nblocks[c] and bd[e] == dd:
                e += 1
            if c == CHN - 1:
                e = min(e, (b // EGRP + 1) * EGRP)
            maxnb = max(SEG_TILES // max(dd, 1), 1)
            nb = min(e - b, maxnb)
            segs.append((c, dd, b, nb, t_off))
            t_off += nb * dd
            b += nb
    return {
        "segs": segs,
        "S_tiles": t_off,
        "nblocks": nblocks,
        "block_deg": block_deg,
    }, per_core, deg


def _per_core_arrays(struct, core_raw, core_deg, edge_weights):
    """Per-core streams: kv idx, ew, q idx, combine idx, npad, orders."""
    o, d, w_idx = core_raw
    ch = np.searchsorted(CH_R0, d, side="right") - 1
    segs = struct["segs"]
    S_tiles = struct["S_tiles"]
    nblocks = struct["nblocks"]
    scale = HD ** -0.5

    orders = []            # slot -> node
    slot_of = []           # node -> slot
    npad = np.zeros(NB, np.float64)
    kvi = np.zeros(S_tiles * 128, np.int32)
    ew4 = np.zeros((S_tiles * 128, H), np.float32)

    for (cc, dd, b0, nb, t_off) in segs:
        kvi[t_off * 128:(t_off + nb * dd) * 128] = CH_ROWS[cc]   # zero row

    for c in range(CHN):
        degc = core_deg[c]
        order = np.argsort(-degc, kind="stable").astype(np.int32)
        inv = np.empty(NB, np.int32)
        inv[order] = np.arange(NB, dtype=np.int32)
        orders.append(order)
        slot_of.append(inv)

        m = ch == c
        oc, dc, wc = o[m], d[m], w_idx[m]
        eorder = np.argsort(inv[oc], kind="stable")
        oc, dc, wc = oc[eorder], dc[eorder], wc[eorder]
        sdeg = degc[order]
        estart = np.zeros(NB + 1, np.int64)
        np.cumsum(sdeg, out=estart[1:])
        ewc = edge_weights[wc] * scale

        for (cc, dd, b0, nb, t_off) in segs:
            if cc != c:
                continue
            for br in range(nb):
                b = b0 + br
                s0 = b * 128
                base = (t_off + br * dd) * 128
                degs = sdeg[s0:s0 + 128]
                nodes = order[s0:s0 + 128]
                npad[nodes] += dd - degs
                for t in range(dd):
                    p = np.nonzero(degs > t)[0]
                    if len(p) == 0:
                        continue
                    ei = estart[s0 + p] + t
                    pos = base + t * 128 + p
                    kvi[pos] = dc[ei] - CH_R0[c]
                    ew4[pos] = ewc[ei]

    ew_t = np.ascontiguousarray(
        ew4.reshape(S_tiles, 128, H).transpose(1, 0, 2)).astype(BF16_NP)
    kvx = _wrap_idx(kvi.astype(np.int16))

    qparts = []
    for c in range(CHN):
        qparts.append(orders[c][:nblocks[c] * 128])
    qix = _wrap_idx(np.concatenate(qparts).astype(np.int16))

    cparts = []
    order3 = orders[CHN - 1]
    for c in range(CHN - 1):
        lim = nblocks[c] * 128
        sc = slot_of[c][order3]
        sc = np.where(sc < lim, sc, PZROW)
        cparts.append(sc)
    cbix = _wrap_idx(np.concatenate(cparts).astype(np.int16))

    npad_t = (npad[order3].astype(np.float32) - 1e-16) \
        .reshape(NBLK, 128).T.copy()

    return {
        "kvx": kvx, "ew": ew_t, "qix": qix, "cbix": cbix,
        "npad": npad_t, "order3": order3,
    }


def _build_graph(struct):
    nc = bacc.Bacc()
    segs = struct["segs"]
    S_tiles = struct["S_tiles"]
    nblocks = struct["nblocks"]

    QCOLS = sum(nblocks) * 8
    CBCOLS = 3 * (NB // 16)

    xT = nc.declare_dram_parameter("xT", [D + 1, NT], BF16, isOutput=False)
    xq = nc.declare_dram_parameter("xq", [D + 1, NB], BF16, isOutput=False)
    wkv = nc.declare_dram_parameter("wkv", [D + 1, 2 * D], BF16,
                                    isOutput=False)
    wq = nc.declare_dram_parameter("wq", [D + 1, D], BF16, isOutput=False)
    wot = nc.declare_dram_parameter("wot", [2 * D, 2 * D], BF16,
                                    isOutput=False)
    boc = nc.declare_dram_parameter("boc", [128, D], BF16, isOutput=False)
    gam = nc.declare_dram_parameter("gam", [128, D], BF16, isOutput=False)
    idnp = nc.declare_dram_parameter("idnp", [128, 128], BF16, isOutput=False)
    xpb = nc.declare_dram_parameter("xpb", [NB, D], BF16, isOutput=False)
    npadp = nc.declare_dram_parameter("npadp", [128, NBLK], F32,
                                      isOutput=False)
    kvxp = nc.declare_dram_parameter("kvx", [128, S_tiles * 8], I16,
                                     isOutput=False)
    ewp = nc.declare_dram_parameter("ewp", [128, S_tiles, H], BF16,
                                    isOutput=False)
    qixp = nc.declare_dram_parameter("qix", [128, QCOLS], I16, isOutput=False)
    cbixp = nc.declare_dram_parameter("cbix", [128, CBCOLS], I16,
                                      isOutput=False)
    out = nc.declare_dram_parameter("out", [NB, D], BF16, isOutput=True)

    kv_tab = [nc.dram_tensor(f"kv_tab{c}", [CH_ROWS[c] + 1, D], U32)
              for c in range(CHN)]
    q_tab = nc.dram_tensor("q_tab", [NB, D], U32)
    partial = [nc.dram_tensor(f"partial{c}", [NB + 1, D], U32)
               for c in range(CHN - 1)]

    qix_off = np.zeros(CHN + 1, np.int64)
    np.cumsum([nblocks[c] * 8 for c in range(CHN)], out=qix_off[1:])
    slab_cum = np.concatenate([[0], np.cumsum(CH_SLABS)])

    with TileContext(nc) as tc:
        with tc.tile_pool(name="const", bufs=1) as cp:
            wkv_t = cp.tile([D + 1, 2 * D], BF16)
            nc.sync.dma_start(out=wkv_t[:], in_=wkv[:])
            wq_t = cp.tile([D + 1, D], BF16)
            nc.sync.dma_start(out=wq_t[:], in_=wq[:])
            wot_t = cp.tile([2 * D, 2 * D], BF16)
            nc.sync.dma_start(out=wot_t[:], in_=wot[:])
            boc_t = cp.tile([128, D], BF16)
            nc.sync.dma_start(out=boc_t[:], in_=boc[:])
            gam_t = cp.tile([128, D], BF16)
            nc.sync.dma_start(out=gam_t[:], in_=gam[:])
            idn_t = cp.tile([128, 128], BF16)
            nc.sync.dma_start(out=idn_t[:], in_=idnp[:])
            npad_t = cp.tile([128, NBLK], F32)
            nc.sync.dma_start(out=npad_t[:], in_=npadp[:])
            acc = cp.tile([128, NBLK, D + H], F32)
            zrow_t = cp.tile([128, D], U32)
            nc.vector.memset(zrow_t[:].bitcast(F32), 0.0)
            for c in range(CHN):
                nc.scalar.dma_start(
                    out=kv_tab[c][CH_ROWS[c]:CH_ROWS[c] + 1, :],
                    in_=zrow_t[0:1, :])
            for c in range(CHN - 1):
                nc.scalar.dma_start(out=partial[c][PZROW:PZROW + 1, :],
                                    in_=zrow_t[0:1, :])

            q_sb = cp.tile([128, 2, NBLK, D], BF16)

            # preload the one activation table set covering Exp/Ln/Copy/Square
            from concourse.hw_specs import get_activation_tables
            _tabs = list(get_activation_tables(nc.m.arch).items())
            _nlx = [i for i, (n, f) in enumerate(_tabs)
                    if n == "natural_log_exp_and_others"][0]
            _atl = mybir.InstLoadActFuncSet(
                name=nc.get_next_instruction_name(), ins=[], outs=[],
                act_func_set_id=_nlx)
            _atl.engine = mybir.EngineType.Activation
            nc.scalar.add_instruction(_atl)

            gp_cm = tc.tile_pool(name="gat", bufs=3)
            gp = gp_cm.__enter__()
            mp_cm = tc.tile_pool(name="met", bufs=2)
            mp = mp_cm.__enter__()
            wp_cm = tc.tile_pool(name="wrk", bufs=2)
            wp = wp_cm.__enter__()
            bp_cm = tc.tile_pool(name="bps", bufs=4, space="PSUM")
            bp = bp_cm.__enter__()
            p1_cm = tc.tile_pool(name="p1sb", bufs=2)
            p1 = p1_cm.__enter__()
            p1p_cm = tc.tile_pool(name="p1ps", bufs=2, space="PSUM")
            p1p = p1p_cm.__enter__()

            slab_no = [0]
            NPRO = 10                 # prologue slabs: q(7) + kv0(5) overlap

            def qkv_slab(src, col0, wt, wcols, tab, row0):
                """One 1792-col slab -> table rows (baseline pattern)."""
                sn = slab_no[0]
                slab_no[0] += 1
                xs = p1.tile([D + 1, SLAB], BF16, tag="xs")
                ((nc.scalar if sn % 2 == 0 else nc.sync)
                 if sn >= 15 else nc.scalar).dma_start(
                    out=xs[:], in_=src[:, col0:col0 + SLAB])
                sb = p1.tile([128, SLAB // 128, wcols], BF16, tag="sb")
                gt = 1024 // wcols
                for g0 in range(0, SLAB // 128, gt):
                    gn = min(gt, SLAB // 128 - g0)
                    ps = p1p.tile([128, 1024], F32, tag="ps")
                    for j in range(gn):
                        jj = g0 + j
                        nc.tensor.matmul(
                            out=ps[:, j * wcols:(j + 1) * wcols],
                            lhsT=xs[:, jj * 128:(jj + 1) * 128],
                            rhs=wt[:], start=True, stop=True)
                    dst = sb[:, g0:g0 + gn, :].rearrange("p a d -> p (a d)")
                    # NOTE: GPSIMD cannot read PSUM on real HW (verifier)
                    if sn < NPRO + 2:
                        eng = nc.vector
                    else:
                        eng = (nc.scalar, nc.scalar, nc.vector)[sn % 3]
                    if eng is nc.scalar:
                        eng.copy(dst, ps[:, 0:gn * wcols])
                    else:
                        eng.tensor_copy(dst, ps[:, 0:gn * wcols])
                nc.sync.dma_start(
                    out=tab[row0:row0 + SLAB, 0:wcols // 2]
                        .bitcast(BF16)
                        .rearrange("(p a) d -> p a d", p=128),
                    in_=sb[:])

            kv_emitted = 0

            def emit_kv_slabs(upto):
                nonlocal kv_emitted
                upto = min(upto, int(slab_cum[-1]))
                while kv_emitted < upto:
                    s = kv_emitted
                    c = int(np.searchsorted(slab_cum, s, side="right")) - 1
                    sl = s - int(slab_cum[c])
                    qkv_slab(xT, int(CH_R0[c]) + sl * SLAB, wkv_t, 2 * D,
                             kv_tab[c], sl * SLAB)
                    kv_emitted += 1

            # prologue: interleave q slabs with kv chunk-0 slabs
            for s in range(NB // SLAB):                  # 7 q slabs
                qkv_slab(xq, s * SLAB, wq_t, D, q_tab, s * SLAB)
                if s % 2 == 1:
                    emit_kv_slabs(kv_emitted + 1)
            emit_kv_slabs(CH_SLABS[0])

            def q_gather(c):
                """Gather chunk-c q rows into q_sb[:, c % 2] (in pieces)."""
                nbk = nblocks[c]
                for h0 in range(0, nbk, SEG_TILES):
                    hn = min(SEG_TILES, nbk - h0)
                    qxi = mp.tile([128, SEG_TILES * 8], I16, tag="qxi")
                    nc.scalar.dma_start(
                        out=qxi[:, 0:hn * 8],
                        in_=qixp[:, qix_off[c] + h0 * 8:
                                 qix_off[c] + (h0 + hn) * 8])
                    qg = gp.tile([128, SEG_TILES, D], U32, tag="kvg")
                    nc.gpsimd.dma_gather(
                        out_ap=qg[:, 0:hn, :], in_ap=q_tab[:],
                        idxs_ap=qxi[:, 0:hn * 8],
                        num_idxs=hn * 128, num_idxs_reg=hn * 128,
                        elem_size=D, single_packet=False)
                    nc.vector.tensor_copy(
                        q_sb[:, c % 2, h0:h0 + hn, :],
                        qg[:, 0:hn, :].bitcast(BF16)[:, :, 0:D])

            q_gather(0)

            # per-chunk batched idx/ew loads; segments slice these tiles
            stream = {"kxi": None, "ew": None, "t0": 0, "tn": 0}

            def load_stream(t0, tn):
                stream["t0"], stream["tn"] = t0, tn
                kxi = mp.tile([128, 224 * 8], I16, tag="kxi")
                nc.sync.dma_start(
                    out=kxi[:, 0:tn * 8],
                    in_=kvxp[:, t0 * 8:(t0 + tn) * 8])
                ew_t = mp.tile([128, 224, H], BF16, tag="ew")
                nc.sync.dma_start(
                    out=ew_t[:, 0:tn, :], in_=ewp[:, t0:t0 + tn, :])
                stream["kxi"], stream["ew"] = kxi, ew_t

            def do_cwindow(c, cwin, cbt=None):
                """One compute window: a run of segments. One gather + one
                merged elementwise chain; qk and the matmul reduction go
                per segment (the q broadcast AP needs uniform degree)."""
                tw0 = cwin[0][4]
                W = cwin[-1][4] + cwin[-1][1] * cwin[-1][3] - tw0
                r0 = tw0 - stream["t0"]
                assert r0 >= 0 and r0 + W <= stream["tn"] and W <= CWIN
                kvg = gp.tile([128, CWIN, D], U32, tag="kvg")
                nc.gpsimd.dma_gather(
                    out_ap=kvg[:, 0:W, :], in_ap=kv_tab[c][:],
                    idxs_ap=stream["kxi"][:, r0 * 8:(r0 + W) * 8],
                    num_idxs=W * 128, num_idxs_reg=W * 128,
                    elem_size=D, single_packet=False)
                ew_t = stream["ew"][:, r0:r0 + W, :]

                kvb = kvg[:, 0:W, :].bitcast(BF16)    # [128, W, 128]
                qk = wp.tile([128, CWIN, D], BF16, tag="qk")
                for (cc, dd, b0, nb, t_off) in cwin:
                    s0 = t_off - tw0
                    T = nb * dd
                    qc = q_sb[:, c % 2, b0:b0 + nb, :] \
                        .rearrange("p b (o d) -> p b o d", o=1) \
                        .to_broadcast([128, nb, dd, D])
                    nc.vector.tensor_tensor(
                        out=qk[:, s0:s0 + T, :]
                            .rearrange("p (b t) d -> p b t d", b=nb),
                        in0=kvb[:, s0:s0 + T, 0:D]
                            .rearrange("p (b t) d -> p b t d", b=nb),
                        in1=qc, op=mybir.AluOpType.mult)

                qk4 = qk[:, 0:W, :].rearrange("p t (h d) -> p t h d", h=H)
                s1 = wp.tile([128, CWIN, 32], BF16, tag="s1")
                s14 = s1[:, 0:W, :].rearrange("p t (h d) -> p t h d", h=H)
                eng_t1 = nc.gpsimd if (TREE1_ON_POOL and c < CHN - 1) \
                    else nc.vector
                eng_t1.tensor_tensor(
                    out=s14, in0=qk4[:, :, :, 0:8], in1=qk4[:, :, :, 8:16],
                    op=mybir.AluOpType.add)
                s2 = wp.tile([128, CWIN, 16], BF16, tag="s2")
                s24 = s2[:, 0:W, :].rearrange("p t (h d) -> p t h d", h=H)
                nc.vector.tensor_tensor(
                    out=s24, in0=s14[:, :, :, 0:4], in1=s14[:, :, :, 4:8],
                    op=mybir.AluOpType.add)
                s3 = wp.tile([128, CWIN, 8], BF16, tag="s3")
                s34 = s3[:, 0:W, :].rearrange("p t (h d) -> p t h d", h=H)
                nc.vector.tensor_tensor(
                    out=s34, in0=s24[:, :, :, 0:2], in1=s24[:, :, :, 2:4],
                    op=mybir.AluOpType.add)
                sc = wp.tile([128, CWIN, H], F32, tag="sc")
                sc4 = sc[:, 0:W, :].rearrange("p t (h o) -> p t h o", h=H)
                (nc.gpsimd if c < CHN - 1 else nc.vector).tensor_tensor(
                    out=sc4, in0=s34[:, :, :, 0:1], in1=s34[:, :, :, 1:2],
                    op=mybir.AluOpType.add)
                ws = wp.tile([128, CWIN, H], F32, tag="ws")
                eng_ws = nc.gpsimd if WS_ON_POOL else nc.vector
                eng_ws.tensor_tensor(
                    out=ws[:, 0:W, :], in0=sc[:, 0:W, :],
                    in1=ew_t, op=mybir.AluOpType.mult)
                wv = wp.tile([128, CWIN, D + H], BF16, tag="wv")
                nc.scalar.activation(
                    out=wv[:, 0:W, D:D + H], in_=ws[:, 0:W, :],
                    func=mybir.ActivationFunctionType.Exp)
                eng_ctb = nc.gpsimd if (CTB_ON_POOL and c == 0) \
                    else nc.vector
                eng_ctb.tensor_tensor(
                    out=wv[:, 0:W, 0:D]
                        .rearrange("p t (e h) -> p t e h", h=H),
                    in0=kvb[:, :, D:2 * D]
                        .rearrange("p t (e h) -> p t e h", h=H),
                    in1=wv[:, 0:W, D:D + H]
                        .rearrange("p t (o h) -> p t o h", o=1)
                        .to_broadcast([128, W, HD, H]),
                    op=mybir.AluOpType.mult)

                for (cc, dd, b0, nb, t_off) in cwin:
                    s0 = t_off - tw0
                    for g0 in range(0, nb, 7):
                        gn = min(7, nb - g0)
                        ncmb = 0 if cbt is None else CHN - 1
                        gcbs = []
                        for ci in range(ncmb):
                            gcb = gp.tile([128, 7, D], U32, tag=f"gcb{ci}")
                            nc.gpsimd.dma_gather(
                                out_ap=gcb[:, 0:gn, :], in_ap=partial[ci][:],
                                idxs_ap=cbt[ci][:, (b0 + g0) * 8:
                                                (b0 + g0 + gn) * 8],
                                num_idxs=gn * 128, num_idxs_reg=gn * 128,
                                elem_size=D, single_packet=False)
                            gcbs.append(gcb)
                        psum = bp.tile([128, 7, D + H], F32, tag="bps")
                        for br in range(gn):
                            for t in range(dd):
                                nc.tensor.matmul(
                                    out=psum[:, br, :], lhsT=idn_t[:],
                                    rhs=wv[:, s0 + (g0 + br) * dd + t, :],
                                    start=(br == 0 and t == 0),
                                    stop=(ncmb == 0 and br == gn - 1
                                          and t == dd - 1))
                        for ci in range(ncmb):
                            gv = gcbs[ci][:].bitcast(BF16)
                            for br in range(gn):
                                nc.tensor.matmul(
                                    out=psum[:, br, :], lhsT=idn_t[:],
                                    rhs=gv[:, br, 0:D + H],
                                    start=False,
                                    stop=(ci == ncmb - 1 and br == gn - 1))
                        if c < CHN - 1:
                            stage = wp.tile([128, 7, 2 * D], BF16, tag="stg")
                            if (b0 + g0) % 2 == 0:
                                nc.scalar.copy(stage[:, 0:gn, 0:D + H],
                                               psum[:, 0:gn, :])
                            else:
                                nc.vector.tensor_copy(
                                    stage[:, 0:gn, 0:D + H],
                                    psum[:, 0:gn, :])
                            nc.sync.dma_start(
                                out=partial[c][(b0 + g0) * 128:
                                               (b0 + g0 + gn) * 128, 0:34]
                                    .rearrange("(a p) d -> p a d", p=128),
                                in_=stage[:, 0:gn, 0:68].bitcast(U32))
                        else:
                            nc.scalar.copy(
                                acc[:, b0 + g0:b0 + g0 + gn, :],
                                psum[:, 0:gn, :])

            def epilogue_group(ep, epp, b0, nbk):
                accs = acc[:, b0:b0 + nbk, :]
                zr = ep.tile([128, EGRP, H], F32, tag="zr")
                nc.vector.tensor_tensor(
                    out=zr[:, 0:nbk, :], in0=accs[:, :, D:D + H],
                    in1=npad_t[:, b0:b0 + nbk]
                        .rearrange("p (b o) -> p b o", o=1)
                        .to_broadcast([128, nbk, H]),
                    op=mybir.AluOpType.subtract)
                # empty/fake rows have z == npad exactly; keep zr finite
                nc.vector.tensor_scalar(
                    out=zr[:, 0:nbk, :], in0=zr[:, 0:nbk, :],
                    scalar1=1e-16, scalar2=None, op0=mybir.AluOpType.max)
                nc.vector.reciprocal(zr[:, 0:nbk, :], zr[:, 0:nbk, :])
                vals = ep.tile([128, EGRP, D], BF16, tag="vals")
                nc.gpsimd.tensor_tensor(
                    out=vals[:, 0:nbk, :]
                        .rearrange("p b (e h) -> p b e h", h=H),
                    in0=accs[:, :, 0:D]
                        .rearrange("p b (e h) -> p b e h", h=H),
                    in1=zr[:, 0:nbk, :]
                        .rearrange("p b (o h) -> p b o h", o=1)
                        .to_broadcast([128, nbk, HD, H]),
                    op=mybir.AluOpType.mult)
                po_sg = ep.tile([128, EGRP, D], BF16, tag="posg")
                npair = (nbk + 1) // 2
                for p0 in range(0, npair, 4):
                    pn = min(4, npair - p0)
                    po = epp.tile([128, 512], F32, tag="po")
                    for pi in range(pn):
                        g = (p0 + pi) * 2
                        pt = epp.tile([128, 128], BF16, tag="pt")
                        nc.tensor.transpose(
                            out=pt[:],
                            in_=vals[:, g:g + 2, :]
                                .rearrange("p a d -> p (a d)"),
                            identity=idn_t[:])
                        vT = ep.tile([128, 128], BF16, tag="vT")
                        nc.scalar.copy(vT[:], pt[:])
                        nc.tensor.matmul(
                            out=po[:, pi * 128:(pi + 1) * 128],
                            lhsT=vT[:], rhs=wot_t[:],
                            start=True, stop=True)
                    nc.scalar.copy(
                        po_sg[:, p0 * 2:p0 * 2 + pn * 2, :]
                            .rearrange("p a d -> p (a d)"),
                        po[:, 0:pn * 128])
                nmu = ep.tile([128, EGRP], F32, tag="nmu")
                nc.vector.tensor_reduce(
                    out=nmu[:, 0:nbk], in_=po_sg[:, 0:nbk, :],
                    axis=mybir.AxisListType.X, op=mybir.AluOpType.add)
                nc.vector.tensor_scalar_mul(nmu[:, 0:nbk], nmu[:, 0:nbk],
                                            -1.0 / D)
                xpb_g = ep.tile([128, EGRP, D], BF16, tag="xpbg")
                nc.scalar.dma_start(
                    out=xpb_g[:, 0:nbk, :],
                    in_=xpb[:].rearrange("(p a) d -> p a d", p=128)
                        [:, b0:b0 + nbk, :])
                ct_g = ep.tile([128, EGRP, D], BF16, tag="ctg")
                cts = ct_g[:, 0:nbk, :]
                nc.gpsimd.tensor_tensor(
                    out=cts, in0=po_sg[:, 0:nbk, :],
                    in1=nmu[:, 0:nbk].rearrange("p (b o) -> p b o", o=1)
                        .to_broadcast([128, nbk, D]),
                    op=mybir.AluOpType.add)
                nc.gpsimd.tensor_tensor(
                    out=cts, in0=cts,
                    in1=boc_t[:].rearrange("p (o d) -> p o d", o=1)
                        .to_broadcast([128, nbk, D]),
                    op=mybir.AluOpType.add)
                sq = ep.tile([128, EGRP, D], F32, tag="sq")
                nc.scalar.square(sq[:, 0:nbk, :], cts)
                vv_t = ep.tile([128, EGRP], F32, tag="vv")
                vv = vv_t[:, 0:nbk]
                nc.vector.tensor_reduce(
                    out=vv, in_=sq[:, 0:nbk, :],
                    axis=mybir.AxisListType.X, op=mybir.AluOpType.add)
                nc.vector.tensor_scalar(
                    out=vv, in0=vv, scalar1=1.0 / D, scalar2=LN_EPS,
                    op0=mybir.AluOpType.mult, op1=mybir.AluOpType.add)
                # rstd = var^-0.5 via exp(-0.5*ln(var)): Ln/Exp/Copy/Square
                # share one activation table set (no ATL thrash, unlike Sqrt)
                lnv = ep.tile([128, EGRP], F32, tag="lnv")
                nc.scalar.activation(
                    out=lnv[:, 0:nbk], in_=vv,
                    func=mybir.ActivationFunctionType.Ln)
                rstd = ep.tile([128, EGRP], F32, tag="rstd")
                nc.scalar.activation(
                    out=rstd[:, 0:nbk], in_=lnv[:, 0:nbk],
                    func=mybir.ActivationFunctionType.Exp, scale=-0.5)
                ot = ep.tile([128, EGRP, D], BF16, tag="ot")
                nc.gpsimd.tensor_tensor(
                    out=ot[:, 0:nbk, :], in0=cts,
                    in1=rstd[:, 0:nbk].rearrange("p (b o) -> p b o", o=1)
                        .to_broadcast([128, nbk, D]),
                    op=mybir.AluOpType.mult)
                nc.gpsimd.tensor_tensor(
                    out=ot[:, 0:nbk, :], in0=ot[:, 0:nbk, :],
                    in1=gam_t[:].rearrange("p (o d) -> p o d", o=1)
                        .to_broadcast([128, nbk, D]),
                    op=mybir.AluOpType.mult)
                nc.gpsimd.tensor_tensor(
                    out=ot[:, 0:nbk, :], in0=ot[:, 0:nbk, :],
                    in1=xpb_g[:, 0:nbk, :], op=mybir.AluOpType.add)
                nc.sync.dma_start(
                    out=out[:].rearrange("(p a) d -> p a d", p=128)
                        [:, b0:b0 + nbk, :],
                    in_=ot[:, 0:nbk, :])

            # ---- phase 2
            segs_by_chunk = [[] for _ in range(CHN)]
            for sg in segs:
                segs_by_chunk[sg[0]].append(sg)

            def windows(csegs, cap=224):
                """Split segments into <=cap-tile windows."""
                wins, cur, tn = [], [], 0
                for sg in csegs:
                    if cur and tn + sg[1] * sg[3] > cap:
                        wins.append(cur)
                        cur, tn = [], 0
                    cur.append(sg)
                    tn += sg[1] * sg[3]
                if cur:
                    wins.append(cur)
                return wins

            # chunks 0..2 with interleaved next-chunk table build
            for c in range(CHN - 1):
                csegs = segs_by_chunk[c]
                nseg = len(csegs)
                si = 0
                for win in windows(csegs):
                    t0 = win[0][4]
                    tn = win[-1][4] + win[-1][1] * win[-1][3] - t0
                    load_stream(t0, tn)
                    for cwin in windows(win, CWIN):
                        si += len(cwin)
                        emit_kv_slabs(int(slab_cum[c + 1])
                                      + (CH_SLABS[c + 1] * si) // nseg)
                        if si >= nseg - 1 and si - len(cwin) < nseg - 1:
                            q_gather(c + 1)
                        do_cwindow(c, cwin)

            # phase-1 pools done; free PSUM banks for the epilogue
            p1p_cm.__exit__(None, None, None)
            p1_cm.__exit__(None, None, None)
            ep_cm = tc.tile_pool(name="ep", bufs=1)
            ep = ep_cm.__enter__()
            epp_cm = tc.tile_pool(name="epps", bufs=2, space="PSUM")
            epp = epp_cm.__enter__()

            # chunk 3: combine folded into psum; epilogue per 14-block group
            cbt = []
            for ci in range(CHN - 1):
                cxi = mp.tile([128, NB // 16], I16, tag=f"cbt{ci}")
                nc.scalar.dma_start(
                    out=cxi[:],
                    in_=cbixp[:, ci * (NB // 16):(ci + 1) * (NB // 16)])
                cbt.append(cxi)
            next_grp = 0
            for win in windows(segs_by_chunk[CHN - 1]):
                t0 = win[0][4]
                tn = win[-1][4] + win[-1][1] * win[-1][3] - t0
                load_stream(t0, tn)
                for cwin in windows(win, CWIN):
                    do_cwindow(CHN - 1, cwin, cbt=cbt)
                    done_b = cwin[-1][2] + cwin[-1][3]
                    while next_grp + EGRP <= done_b:
                        epilogue_group(ep, epp, next_grp, EGRP)
                        next_grp += EGRP
            while next_grp < NBLK:
                nbk = min(EGRP, NBLK - next_grp)
                epilogue_group(ep, epp, next_grp, nbk)
                next_grp += nbk

            ep_cm.__exit__(None, None, None)
            epp_cm.__exit__(None, None, None)
            bp_cm.__exit__(None, None, None)
            wp_cm.__exit__(None, None, None)
            mp_cm.__exit__(None, None, None)
            gp_cm.__exit__(None, None, None)
    return nc


def kernel(x, edge_index, edge_weights, Wq, bq, Wk, bk, Wv, bv, Wo, bo,
           gamma, beta):
    x = np.asarray(x, np.float32)
    edge_weights = np.asarray(edge_weights, np.float32)
    origins = np.asarray(edge_index[0], np.int64)
    dests = np.asarray(edge_index[1], np.int64)

    struct, per_core, deg = _build_structure(origins, dests)
    nc = _build_graph(struct)
    nc.finalize()

    perm_t = _slab_perm(NT)
    xpad = np.zeros((NT, D), np.float32)
    xpad[:N] = x
    xT = np.empty((D + 1, NT), np.float32)
    xT[:D] = xpad[perm_t].T
    xT[D] = 1.0
    xT = xT.astype(BF16_NP)

    vperm = (np.arange(H)[None, :] * HD + np.arange(HD)[:, None]).ravel()
    wkv = np.zeros((D + 1, 2 * D), np.float32)
    wkv[:D, :D] = np.asarray(Wk, np.float32).T
    wkv[:D, D:] = np.asarray(Wv, np.float32).T[:, vperm]
    wkv[D, :D] = np.asarray(bk, np.float32)
    wkv[D, D:] = np.asarray(bv, np.float32)[vperm]
    wq = np.zeros((D + 1, D), np.float32)
    wq[:D] = np.asarray(Wq, np.float32).T
    wq[D] = np.asarray(bq, np.float32)
    wot1 = np.ascontiguousarray(np.asarray(Wo, np.float32).T[vperm, :])
    wot = np.zeros((2 * D, 2 * D), np.float32)     # block-diagonal pair form
    wot[:D, :D] = wot1
    wot[D:, D:] = wot1
    bo = np.asarray(bo, np.float32)
    boc = np.tile((bo - bo.mean())[None, :], (128, 1))
    gam_t = np.tile(np.asarray(gamma, np.float32)[None, :], (128, 1))
    idn = np.eye(128, dtype=np.float32)
    beta = np.asarray(beta, np.float32)
    perm_q = _slab_perm(NB)

    in_maps = []
    outs_meta = []
    for ci in range(NCORES):
        data = _per_core_arrays(struct, per_core[ci], deg[ci], edge_weights)
        xo = np.zeros((NB, D), np.float32)
        xo[:NOWN] = x[ci * NOWN:(ci + 1) * NOWN]
        xq_c = np.empty((D + 1, NB), np.float32)
        xq_c[:D] = xo[perm_q].T
        xq_c[D] = 1.0
        order3 = data["order3"]
        xpb_c = (xo[order3] + beta[None, :]).reshape(NBLK, 128, D) \
            .transpose(1, 0, 2).reshape(NB, D)
        in_maps.append({
            "xT": xT, "xq": xq_c.astype(BF16_NP),
            "wkv": wkv.astype(BF16_NP), "wq": wq.astype(BF16_NP),
            "wot": wot.astype(BF16_NP), "boc": boc.astype(BF16_NP),
            "gam": gam_t.astype(BF16_NP), "idnp": idn.astype(BF16_NP),
            "xpb": xpb_c.astype(BF16_NP), "npadp": data["npad"],
            "kvx": data["kvx"], "ewp": data["ew"],
            "qix": data["qix"], "cbix": data["cbix"],
        })
        outs_meta.append(order3)

    global LAST_SIM_NS
    if SIMULATE_COST:
        from concourse import bass_interp
        sim = bass_interp.CoreSim(nc, no_exec=True, publish_trace=False)
        sim.event_loop()
        LAST_SIM_NS = int(sim.time)

    res = run_bass_kernel_spmd(nc, in_maps, core_ids=list(range(NCORES)),
                               trace=TRACE)
    global LAST_RESULT
    LAST_RESULT = res
    outs = []
    for ci in range(NCORES):
        o = np.asarray(res.results[ci]["out"]).astype(np.float32)
        o = o.reshape(128, NBLK, D).transpose(1, 0, 2).reshape(NB, D)
        inv = np.empty(NB, np.int64)
        inv[outs_meta[ci]] = np.arange(NB)
        outs.append(o[inv[:NOWN]])
    return np.concatenate(outs, axis=0)


TRACE = False
SIMULATE_COST = False
LAST_RESULT = None
LAST_SIM_NS = None


# revision 73
# speedup vs baseline: 1.0117x; 1.0117x over previous
"""Trainium2 Bass kernel for BaseDependentAttentionLayer (GNN message passing).

v3 design (8 NeuronCores, SPMD), structured-slot layout:
  - Edges sharded by origin core. Within a core, each of 4 dest-chunks gets its
    OWN degree-sorted origin permutation: chunk-c slot (block b, partition p)
    holds one origin; tile t of block b holds the t-th chunk-c edge of each
    origin in the block (blocks padded to a uniform per-block degree).
  - Consequences: q is a per-partition broadcast from an SBUF table (no
    per-edge q gather); the scatter-reduction matmul uses a CONSTANT identity
    lhsT (no per-edge one-hot gather). Only ONE 256B gather per edge (k|v).
  - Chunk sizes are uneven ([8,16,17,15] slabs): a small chunk 0 shortens the
    prologue (table build) critical path; a smaller chunk 3 shortens the tail.
  - Per-chunk partials (vals|z) drain to DRAM rows [z f32 | vals bf16] 256B;
    chunk 3 drains straight into the SBUF accumulator, and the combine
    (3 per-node gathers + adds) plus the whole epilogue run interleaved with
    the chunk-3 pass per 14-block group.
  - Softmax pad slots hit a zero k|v row with ew=0 so they add exactly 1.0 to
    z; a host-computed npad tile subtracts them in the epilogue.
"""

import sys

sys.path.insert(0, "/opt/trn_rl_repo")

import numpy as np
import ml_dtypes

import concourse.bass as bass
import concourse.bacc as bacc
import concourse.mybir as mybir
from concourse.tile import TileContext
from concourse.bass_utils import run_bass_kernel_spmd

N = 100000
E = 1600000
D = 64
H = 4
HD = 16
NCORES = 8
NOWN = 12500            # nodes owned per core
NBLK = 98               # 128-node blocks per core (final order)
NB = NBLK * 128         # 12544 padded own nodes
SLAB = 1792
CH_SLABS = [8, 16, 17, 15]
CH_ROWS = [s * SLAB for s in CH_SLABS]
CH_R0 = np.concatenate([[0], np.cumsum(CH_ROWS)])   # len 5
CHN = 4
NT = int(CH_R0[-1])     # 100352
PZROW = NB              # zero row index within each partial table
SEG_TILES = 56          # max tiles per equal-degree segment
CWIN = 64               # compute-window tiles (merged elementwise ops)
EGRP = 14               # epilogue block-group size
LN_EPS = 1e-5

F32 = mybir.dt.float32
BF16 = mybir.dt.bfloat16
U32 = mybir.dt.uint32
I16 = mybir.dt.int16
BF16_NP = ml_dtypes.bfloat16

# engine knobs (tuned against the CoreSim cost model)
WS_ON_POOL = True
TREE1_ON_POOL = True
CTB_ON_POOL = False


def _wrap_idx(vals):
    """SWDGE index layout: [16, n/16] wrapped, replicated to 128 partitions."""
    assert len(vals) % 16 == 0
    w = vals.reshape(-1, 16).T.astype(np.int16)
    return np.tile(w, (8, 1))


def _slab_perm(nrows):
    """Column permutation making table-write DMAs contiguous (baseline)."""
    assert nrows % SLAB == 0
    j = np.arange(nrows)
    s, r = j // SLAB, j % SLAB
    a, p = r // 128, r % 128
    return s * SLAB + p * (SLAB // 128) + a


def _build_structure(origins, dests):
    """Common (cross-core max) per-chunk block-degree structure + per-core
    degree/sort data."""
    owner = origins // NOWN
    per_core = []
    for c in range(NCORES):
        m = owner == c
        o = (origins[m] - c * NOWN).astype(np.int32)
        d = dests[m].astype(np.int32)
        w_idx = np.nonzero(m)[0]
        per_core.append((o, d, w_idx))

    deg = np.zeros((NCORES, CHN, NB), np.int64)
    for ci, (o, d, w_idx) in enumerate(per_core):
        ch = np.searchsorted(CH_R0, d, side="right") - 1
        for c in range(CHN):
            deg[ci, c] = np.bincount(o[ch == c], minlength=NB)

    sdeg = -np.sort(-deg, axis=2)            # per-core sorted degree curves
    common = sdeg.max(axis=0)                # [CHN, NB] common slot degrees

    nblocks = []
    block_deg = []
    for c in range(CHN):
        if c < CHN - 1:
            nz = int((common[c] > 0).sum())
            nb = max((nz + 127) // 128, 1)
        else:
            nb = NBLK
        bd = common[c].reshape(NBLK, 128)[:nb, 0].copy()   # block max = first
        if c == CHN - 1:
            bd = np.maximum(bd, 1)           # every final block drains
        nblocks.append(nb)
        block_deg.append(bd.astype(np.int64))

    # segments: runs of equal block degree, capped at SEG_TILES tiles.
    # chunk-3 segments must not straddle EGRP boundaries (epilogue groups).
    segs = []          # (chunk, d, b0, nb, tile_off)
    t_off = 0
    for c in range(CHN):
        bd = block_deg[c]
        b = 0
        while b < nblocks[c]:
            dd = int(bd[b])
            e = b
            while e < nblocks[c] and bd[e] == dd:
                e += 1
            if c == CHN - 1:
                e = min(e, (b // EGRP + 1) * EGRP)
            maxnb = max(SEG_TILES // max(dd, 1), 1)
            nb = min(e - b, maxnb)
            segs.append((c, dd, b, nb, t_off))
            t_off += nb * dd
            b += nb
    return {
        "segs": segs,
        "S_tiles": t_off,
        "nblocks": nblocks,
        "block_deg": block_deg,
    }, per_core, deg


def _per_core_arrays(struct, core_raw, core_deg, edge_weights):
    """Per-core streams: kv idx, ew, q idx, combine idx, npad, orders."""
    o, d, w_idx = core_raw
    ch = np.searchsorted(CH_R0, d, side="right") - 1
    segs = struct["segs"]
    S_tiles = struct["S_tiles"]
    nblocks = struct["nblocks"]
    scale = HD ** -0.5

    orders = []            # slot -> node
    slot_of = []           # node -> slot
    npad = np.zeros(NB, np.float64)
    kvi = np.zeros(S_tiles * 128, np.int32)
    ew4 = np.zeros((S_tiles * 128, H), np.float32)

    for (cc, dd, b0, nb, t_off) in segs:
        kvi[t_off * 128:(t_off + nb * dd) * 128] = CH_ROWS[cc]   # zero row

    for c in range(CHN):
        degc = core_deg[c]
        order = np.argsort(-degc, kind="stable").astype(np.int32)
        inv = np.empty(NB, np.int32)
        inv[order] = np.arange(NB, dtype=np.int32)
        orders.append(order)
        slot_of.append(inv)

        m = ch == c
        oc, dc, wc = o[m], d[m], w_idx[m]
        eorder = np.argsort(inv[oc], kind="stable")
        oc, dc, wc = oc[eorder], dc[eorder], wc[eorder]
        sdeg = degc[order]
        estart = np.zeros(NB + 1, np.int64)
        np.cumsum(sdeg, out=estart[1:])
        ewc = edge_weights[wc] * scale

        for (cc, dd, b0, nb, t_off) in segs:
            if cc != c:
                continue
            for br in range(nb):
                b = b0 + br
                s0 = b * 128
                base = (t_off + br * dd) * 128
                degs = sdeg[s0:s0 + 128]
                nodes = order[s0:s0 + 128]
                npad[nodes] += dd - degs
                for t in range(dd):
                    p = np.nonzero(degs > t)[0]
                    if len(p) == 0:
                        continue
                    ei = estart[s0 + p] + t
                    pos = base + t * 128 + p
                    kvi[pos] = dc[ei] - CH_R0[c]
                    ew4[pos] = ewc[ei]

    ew_t = np.ascontiguousarray(
        ew4.reshape(S_tiles, 128, H).transpose(1, 0, 2)).astype(BF16_NP)
    kvx = _wrap_idx(kvi.astype(np.int16))

    qparts = []
    for c in range(CHN):
        qparts.append(orders[c][:nblocks[c] * 128])
    qix = _wrap_idx(np.concatenate(qparts).astype(np.int16))

    cparts = []
    order3 = orders[CHN - 1]
    for c in range(CHN - 1):
        lim = nblocks[c] * 128
        sc = slot_of[c][order3]
        sc = np.where(sc < lim, sc, PZROW)
        cparts.append(sc)
    cbix = _wrap_idx(np.concatenate(cparts).astype(np.int16))

    npad_t = (npad[order3].astype(np.float32) - 1e-16) \
        .reshape(NBLK, 128).T.copy()

    return {
        "kvx": kvx, "ew": ew_t, "qix": qix, "cbix": cbix,
        "npad": npad_t, "order3": order3,
    }


def _build_graph(struct):
    nc = bacc.Bacc()
    segs = struct["segs"]
    S_tiles = struct["S_tiles"]
    nblocks = struct["nblocks"]

    QCOLS = sum(nblocks) * 8
    CBCOLS = 3 * (NB // 16)

    xT = nc.declare_dram_parameter("xT", [D + 1, NT], BF16, isOutput=False)
    xq = nc.declare_dram_parameter("xq", [D + 1, NB], BF16, isOutput=False)
    wkv = nc.declare_dram_parameter("wkv", [D + 1, 2 * D], BF16,
                                    isOutput=False)
    wq = nc.declare_dram_parameter("wq", [D + 1, D], BF16, isOutput=False)
    wot = nc.declare_dram_parameter("wot", [2 * D, 2 * D], BF16,
                                    isOutput=False)
    boc = nc.declare_dram_parameter("boc", [128, D], BF16, isOutput=False)
    gam = nc.declare_dram_parameter("gam", [128, D], BF16, isOutput=False)
    idnp = nc.declare_dram_parameter("idnp", [128, 128], BF16, isOutput=False)
    xpb = nc.declare_dram_parameter("xpb", [NB, D], BF16, isOutput=False)
    npadp = nc.declare_dram_parameter("npadp", [128, NBLK], F32,
                                      isOutput=False)
    kvxp = nc.declare_dram_parameter("kvx", [128, S_tiles * 8], I16,
                                     isOutput=False)
    ewp = nc.declare_dram_parameter("ewp", [128, S_tiles, H], BF16,
                                    isOutput=False)
    qixp = nc.declare_dram_parameter("qix", [128, QCOLS], I16, isOutput=False)
    cbixp = nc.declare_dram_parameter("cbix", [128, CBCOLS], I16,
                                      isOutput=False)
    out = nc.declare_dram_parameter("out", [NB, D], BF16, isOutput=True)

    kv_tab = [nc.dram_tensor(f"kv_tab{c}", [CH_ROWS[c] + 1, D], U32)
              for c in range(CHN)]
    q_tab = nc.dram_tensor("q_tab", [NB, D], U32)
    partial = [nc.dram_tensor(f"partial{c}", [NB + 1, D], U32)
               for c in range(CHN - 1)]

    qix_off = np.zeros(CHN + 1, np.int64)
    np.cumsum([nblocks[c] * 8 for c in range(CHN)], out=qix_off[1:])
    slab_cum = np.concatenate([[0], np.cumsum(CH_SLABS)])

    with TileContext(nc) as tc:
        with tc.tile_pool(name="const", bufs=1) as cp:
            wkv_t = cp.tile([D + 1, 2 * D], BF16)
            nc.sync.dma_start(out=wkv_t[:], in_=wkv[:])
            wq_t = cp.tile([D + 1, D], BF16)
            nc.sync.dma_start(out=wq_t[:], in_=wq[:])
            wot_t = cp.tile([2 * D, 2 * D], BF16)
            nc.sync.dma_start(out=wot_t[:], in_=wot[:])
            boc_t = cp.tile([128, D], BF16)
            nc.sync.dma_start(out=boc_t[:], in_=boc[:])
            gam_t = cp.tile([128, D], BF16)
            nc.sync.dma_start(out=gam_t[:], in_=gam[:])
            idn_t = cp.tile([128, 128], BF16)
            nc.sync.dma_start(out=idn_t[:], in_=idnp[:])
            npad_t = cp.tile([128, NBLK], F32)
            nc.sync.dma_start(out=npad_t[:], in_=npadp[:])
            acc = cp.tile([128, NBLK, D + H], F32)
            zrow_t = cp.tile([128, D], U32)
            nc.vector.memset(zrow_t[:].bitcast(F32), 0.0)
            for c in range(CHN):
                nc.scalar.dma_start(
                    out=kv_tab[c][CH_ROWS[c]:CH_ROWS[c] + 1, :],
                    in_=zrow_t[0:1, :])
            for c in range(CHN - 1):
                nc.scalar.dma_start(out=partial[c][PZROW:PZROW + 1, :],
                                    in_=zrow_t[0:1, :])

            q_sb = cp.tile([128, 2, NBLK, D], BF16)

            # preload the one activation table set covering Exp/Ln/Copy/Square
            from concourse.hw_specs import get_activation_tables
            _tabs = list(get_activation_tables(nc.m.arch).items())
            _nlx = [i for i, (n, f) in enumerate(_tabs)
                    if n == "natural_log_exp_and_others"][0]
            _atl = mybir.InstLoadActFuncSet(
                name=nc.get_next_instruction_name(), ins=[], outs=[],
                act_func_set_id=_nlx)
            _atl.engine = mybir.EngineType.Activation
            nc.scalar.add_instruction(_atl)

            gp_cm = tc.tile_pool(name="gat", bufs=3)
            gp = gp_cm.__enter__()
            mp_cm = tc.tile_pool(name="met", bufs=2)
            mp = mp_cm.__enter__()
            wp_cm = tc.tile_pool(name="wrk", bufs=2)
            wp = wp_cm.__enter__()
            bp_cm = tc.tile_pool(name="bps", bufs=4, space="PSUM")
            bp = bp_cm.__enter__()
            p1_cm = tc.tile_pool(name="p1sb", bufs=2)
            p1 = p1_cm.__enter__()
            p1p_cm = tc.tile_pool(name="p1ps", bufs=2, space="PSUM")
            p1p = p1p_cm.__enter__()

            slab_no = [0]
            NPRO = 10                 # prologue slabs: q(7) + kv0(5) overlap

            def qkv_slab(src, col0, wt, wcols, tab, row0):
                """One 1792-col slab -> table rows (baseline pattern)."""
                sn = slab_no[0]
                slab_no[0] += 1
                xs = p1.tile([D + 1, SLAB], BF16, tag="xs")
                ((nc.scalar if sn % 2 == 0 else nc.sync)
                 if sn >= 15 else nc.scalar).dma_start(
                    out=xs[:], in_=src[:, col0:col0 + SLAB])
                sb = p1.tile([128, SLAB // 128, wcols], BF16, tag="sb")
                gt = 1024 // wcols
                for g0 in range(0, SLAB // 128, gt):
                    gn = min(gt, SLAB // 128 - g0)
                    ps = p1p.tile([128, 1024], F32, tag="ps")
                    for j in range(gn):
                        jj = g0 + j
                        nc.tensor.matmul(
                            out=ps[:, j * wcols:(j + 1) * wcols],
                            lhsT=xs[:, jj * 128:(jj + 1) * 128],
                            rhs=wt[:], start=True, stop=True)
                    dst = sb[:, g0:g0 + gn, :].rearrange("p a d -> p (a d)")
                    # NOTE: GPSIMD cannot read PSUM on real HW (verifier)
                    if sn < NPRO + 2:
                        eng = nc.vector
                    else:
                        eng = (nc.scalar, nc.scalar, nc.vector)[sn % 3]
                    if eng is nc.scalar:
                        eng.copy(dst, ps[:, 0:gn * wcols])
                    else:
                        eng.tensor_copy(dst, ps[:, 0:gn * wcols])
                nc.sync.dma_start(
                    out=tab[row0:row0 + SLAB, 0:wcols // 2]
                        .bitcast(BF16)
                        .rearrange("(p a) d -> p a d", p=128),
                    in_=sb[:])

            kv_emitted = 0

            def emit_kv_slabs(upto):
                nonlocal kv_emitted
                upto = min(upto, int(slab_cum[-1]))
                while kv_emitted < upto:
                    s = kv_emitted
                    c = int(np.searchsorted(slab_cum, s, side="right")) - 1
                    sl = s - int(slab_cum[c])
                    qkv_slab(xT, int(CH_R0[c]) + sl * SLAB, wkv_t, 2 * D,
                             kv_tab[c], sl * SLAB)
                    kv_emitted += 1

            # prologue: interleave q slabs with kv chunk-0 slabs
            for s in range(NB // SLAB):                  # 7 q slabs
                qkv_slab(xq, s * SLAB, wq_t, D, q_tab, s * SLAB)
                if s % 2 == 1:
                    emit_kv_slabs(kv_emitted + 1)
            emit_kv_slabs(CH_SLABS[0])

            def q_gather(c):
                """Gather chunk-c q rows into q_sb[:, c % 2] (in pieces)."""
                nbk = nblocks[c]
                for h0 in range(0, nbk, SEG_TILES):
                    hn = min(SEG_TILES, nbk - h0)
                    qxi = mp.tile([128, SEG_TILES * 8], I16, tag="qxi")
                    nc.scalar.dma_start(
                        out=qxi[:, 0:hn * 8],
                        in_=qixp[:, qix_off[c] + h0 * 8:
                                 qix_off[c] + (h0 + hn) * 8])
                    qg = gp.tile([128, SEG_TILES, D], U32, tag="kvg")
                    nc.gpsimd.dma_gather(
                        out_ap=qg[:, 0:hn, :], in_ap=q_tab[:],
                        idxs_ap=qxi[:, 0:hn * 8],
                        num_idxs=hn * 128, num_idxs_reg=hn * 128,
                        elem_size=D, single_packet=False)
                    nc.vector.tensor_copy(
                        q_sb[:, c % 2, h0:h0 + hn, :],
                        qg[:, 0:hn, :].bitcast(BF16)[:, :, 0:D])

            q_gather(0)

            # per-chunk batched idx/ew loads; segments slice these tiles
            stream = {"kxi": None, "ew": None, "t0": 0, "tn": 0}

            def load_stream(t0, tn):
                stream["t0"], stream["tn"] = t0, tn
                kxi = mp.tile([128, 256 * 8], I16, tag="kxi")
                nc.sync.dma_start(
                    out=kxi[:, 0:tn * 8],
                    in_=kvxp[:, t0 * 8:(t0 + tn) * 8])
                ew_t = mp.tile([128, 256, H], BF16, tag="ew")
                nc.sync.dma_start(
                    out=ew_t[:, 0:tn, :], in_=ewp[:, t0:t0 + tn, :])
                stream["kxi"], stream["ew"] = kxi, ew_t

            def do_cwindow(c, cwin, cbt=None):
                """One compute window: a run of segments. One gather + one
                merged elementwise chain; qk and the matmul reduction go
                per segment (the q broadcast AP needs uniform degree)."""
                tw0 = cwin[0][4]
                W = cwin[-1][4] + cwin[-1][1] * cwin[-1][3] - tw0
                r0 = tw0 - stream["t0"]
                assert r0 >= 0 and r0 + W <= stream["tn"] and W <= CWIN
                kvg = gp.tile([128, CWIN, D], U32, tag="kvg")
                nc.gpsimd.dma_gather(
                    out_ap=kvg[:, 0:W, :], in_ap=kv_tab[c][:],
                    idxs_ap=stream["kxi"][:, r0 * 8:(r0 + W) * 8],
                    num_idxs=W * 128, num_idxs_reg=W * 128,
                    elem_size=D, single_packet=False)
                ew_t = stream["ew"][:, r0:r0 + W, :]

                kvb = kvg[:, 0:W, :].bitcast(BF16)    # [128, W, 128]
                qk = wp.tile([128, CWIN, D], BF16, tag="qk")
                for (cc, dd, b0, nb, t_off) in cwin:
                    s0 = t_off - tw0
                    T = nb * dd
                    qc = q_sb[:, c % 2, b0:b0 + nb, :] \
                        .rearrange("p b (o d) -> p b o d", o=1) \
                        .to_broadcast([128, nb, dd, D])
                    nc.vector.tensor_tensor(
                        out=qk[:, s0:s0 + T, :]
                            .rearrange("p (b t) d -> p b t d", b=nb),
                        in0=kvb[:, s0:s0 + T, 0:D]
                            .rearrange("p (b t) d -> p b t d", b=nb),
                        in1=qc, op=mybir.AluOpType.mult)

                qk4 = qk[:, 0:W, :].rearrange("p t (h d) -> p t h d", h=H)
                s1 = wp.tile([128, CWIN, 32], BF16, tag="s1")
                s14 = s1[:, 0:W, :].rearrange("p t (h d) -> p t h d", h=H)
                eng_t1 = nc.gpsimd if (TREE1_ON_POOL and c < CHN - 1) \
                    else nc.vector
                eng_t1.tensor_tensor(
                    out=s14, in0=qk4[:, :, :, 0:8], in1=qk4[:, :, :, 8:16],
                    op=mybir.AluOpType.add)
                s2 = wp.tile([128, CWIN, 16], BF16, tag="s2")
                s24 = s2[:, 0:W, :].rearrange("p t (h d) -> p t h d", h=H)
                nc.vector.tensor_tensor(
                    out=s24, in0=s14[:, :, :, 0:4], in1=s14[:, :, :, 4:8],
                    op=mybir.AluOpType.add)
                s3 = wp.tile([128, CWIN, 8], BF16, tag="s3")
                s34 = s3[:, 0:W, :].rearrange("p t (h d) -> p t h d", h=H)
                nc.vector.tensor_tensor(
                    out=s34, in0=s24[:, :, :, 0:2], in1=s24[:, :, :, 2:4],
                    op=mybir.AluOpType.add)
                sc = wp.tile([128, CWIN, H], F32, tag="sc")
                sc4 = sc[:, 0:W, :].rearrange("p t (h o) -> p t h o", h=H)
                (nc.gpsimd if c < CHN - 1 else nc.vector).tensor_tensor(
                    out=sc4, in0=s34[:, :, :, 0:1], in1=s34[:, :, :, 1:2],
                    op=mybir.AluOpType.add)
                ws = wp.tile([128, CWIN, H], F32, tag="ws")
                eng_ws = nc.gpsimd if WS_ON_POOL else nc.vector
                eng_ws.tensor_tensor(
                    out=ws[:, 0:W, :], in0=sc[:, 0:W, :],
                    in1=ew_t, op=mybir.AluOpType.mult)
                wv = wp.tile([128, CWIN, D + H], BF16, tag="wv")
                nc.scalar.activation(
                    out=wv[:, 0:W, D:D + H], in_=ws[:, 0:W, :],
                    func=mybir.ActivationFunctionType.Exp)
                eng_ctb = nc.gpsimd if (CTB_ON_POOL and c == 0) \
                    else nc.vector
                eng_ctb.tensor_tensor(
                    out=wv[:, 0:W, 0:D]
                        .rearrange("p t (e h) -> p t e h", h=H),
                    in0=kvb[:, :, D:2 * D]
                        .rearrange("p t (e h) -> p t e h", h=H),
                    in1=wv[:, 0:W, D:D + H]
                        .rearrange("p t (o h) -> p t o h", o=1)
                        .to_broadcast([128, W, HD, H]),
                    op=mybir.AluOpType.mult)

                for (cc, dd, b0, nb, t_off) in cwin:
                    s0 = t_off - tw0
                    for g0 in range(0, nb, 7):
                        gn = min(7, nb - g0)
                        ncmb = 0 if cbt is None else CHN - 1
                        gcbs = []
                        for ci in range(ncmb):
                            gcb = gp.tile([128, 7, D], U32, tag=f"gcb{ci}")
                            nc.gpsimd.dma_gather(
                                out_ap=gcb[:, 0:gn, :], in_ap=partial[ci][:],
                                idxs_ap=cbt[ci][:, (b0 + g0) * 8:
                                                (b0 + g0 + gn) * 8],
                                num_idxs=gn * 128, num_idxs_reg=gn * 128,
                                elem_size=D, single_packet=False)
                            gcbs.append(gcb)
                        psum = bp.tile([128, 7, D + H], F32, tag="bps")
                        for br in range(gn):
                            for t in range(dd):
                                nc.tensor.matmul(
                                    out=psum[:, br, :], lhsT=idn_t[:],
                                    rhs=wv[:, s0 + (g0 + br) * dd + t, :],
                                    start=(br == 0 and t == 0),
                                    stop=(ncmb == 0 and br == gn - 1
                                          and t == dd - 1))
                        for ci in range(ncmb):
                            gv = gcbs[ci][:].bitcast(BF16)
                            for br in range(gn):
                                nc.tensor.matmul(
                                    out=psum[:, br, :], lhsT=idn_t[:],
                                    rhs=gv[:, br, 0:D + H],
                                    start=False,
                                    stop=(ci == ncmb - 1 and br == gn - 1))
                        if c < CHN - 1:
                            stage = wp.tile([128, 7, 2 * D], BF16, tag="stg")
                            if (b0 + g0) % 2 == 0:
                                nc.scalar.copy(stage[:, 0:gn, 0:D + H],
                                               psum[:, 0:gn, :])
                            else:
                                nc.vector.tensor_copy(
                                    stage[:, 0:gn, 0:D + H],
                                    psum[:, 0:gn, :])
                            nc.sync.dma_start(
                                out=partial[c][(b0 + g0) * 128:
                                               (b0 + g0 + gn) * 128, 0:34]
                                    .rearrange("(a p) d -> p a d", p=128),
                                in_=stage[:, 0:gn, 0:68].bitcast(U32))
                        else:
                            nc.scalar.copy(
                                acc[:, b0 + g0:b0 + g0 + gn, :],
                                psum[:, 0:gn, :])

            def epilogue_group(ep, epp, b0, nbk):
                accs = acc[:, b0:b0 + nbk, :]
                zr = ep.tile([128, EGRP, H], F32, tag="zr")
                nc.vector.tensor_tensor(
                    out=zr[:, 0:nbk, :], in0=accs[:, :, D:D + H],
                    in1=npad_t[:, b0:b0 + nbk]
                        .rearrange("p (b o) -> p b o", o=1)
                        .to_broadcast([128, nbk, H]),
                    op=mybir.AluOpType.subtract)
                # empty/fake rows have z == npad exactly; keep zr finite
                nc.vector.tensor_scalar(
                    out=zr[:, 0:nbk, :], in0=zr[:, 0:nbk, :],
                    scalar1=1e-16, scalar2=None, op0=mybir.AluOpType.max)
                nc.vector.reciprocal(zr[:, 0:nbk, :], zr[:, 0:nbk, :])
                vals = ep.tile([128, EGRP, D], BF16, tag="vals")
                nc.gpsimd.tensor_tensor(
                    out=vals[:, 0:nbk, :]
                        .rearrange("p b (e h) -> p b e h", h=H),
                    in0=accs[:, :, 0:D]
                        .rearrange("p b (e h) -> p b e h", h=H),
                    in1=zr[:, 0:nbk, :]
                        .rearrange("p b (o h) -> p b o h", o=1)
                        .to_broadcast([128, nbk, HD, H]),
                    op=mybir.AluOpType.mult)
                po_sg = ep.tile([128, EGRP, D], BF16, tag="posg")
                npair = (nbk + 1) // 2
                for p0 in range(0, npair, 4):
                    pn = min(4, npair - p0)
                    po = epp.tile([128, 512], F32, tag="po")
                    for pi in range(pn):
                        g = (p0 + pi) * 2
                        pt = epp.tile([128, 128], BF16, tag="pt")
                        nc.tensor.transpose(
                            out=pt[:],
                            in_=vals[:, g:g + 2, :]
                                .rearrange("p a d -> p (a d)"),
                            identity=idn_t[:])
                        vT = ep.tile([128, 128], BF16, tag="vT")
                        nc.scalar.copy(vT[:], pt[:])
                        nc.tensor.matmul(
                            out=po[:, pi * 128:(pi + 1) * 128],
                            lhsT=vT[:], rhs=wot_t[:],
                            start=True, stop=True)
                    nc.scalar.copy(
                        po_sg[:, p0 * 2:p0 * 2 + pn * 2, :]
                            .rearrange("p a d -> p (a d)"),
                        po[:, 0:pn * 128])
                nmu = ep.tile([128, EGRP], F32, tag="nmu")
                nc.vector.tensor_reduce(
                    out=nmu[:, 0:nbk], in_=po_sg[:, 0:nbk, :],
                    axis=mybir.AxisListType.X, op=mybir.AluOpType.add)
                nc.vector.tensor_scalar_mul(nmu[:, 0:nbk], nmu[:, 0:nbk],
                                            -1.0 / D)
                xpb_g = ep.tile([128, EGRP, D], BF16, tag="xpbg")
                nc.scalar.dma_start(
                    out=xpb_g[:, 0:nbk, :],
                    in_=xpb[:].rearrange("(p a) d -> p a d", p=128)
                        [:, b0:b0 + nbk, :])
                ct_g = ep.tile([128, EGRP, D], BF16, tag="ctg")
                cts = ct_g[:, 0:nbk, :]
                nc.gpsimd.tensor_tensor(
                    out=cts, in0=po_sg[:, 0:nbk, :],
                    in1=nmu[:, 0:nbk].rearrange("p (b o) -> p b o", o=1)
                        .to_broadcast([128, nbk, D]),
                    op=mybir.AluOpType.add)
                nc.gpsimd.tensor_tensor(
                    out=cts, in0=cts,
                    in1=boc_t[:].rearrange("p (o d) -> p o d", o=1)
                        .to_broadcast([128, nbk, D]),
                    op=mybir.AluOpType.add)
                sq = ep.tile([128, EGRP, D], F32, tag="sq")
                nc.scalar.square(sq[:, 0:nbk, :], cts)
                vv_t = ep.tile([128, EGRP], F32, tag="vv")
                vv = vv_t[:, 0:nbk]
                nc.vector.tensor_reduce(
                    out=vv, in_=sq[:, 0:nbk, :],
                    axis=mybir.AxisListType.X, op=mybir.AluOpType.add)
                nc.vector.tensor_scalar(
                    out=vv, in0=vv, scalar1=1.0 / D, scalar2=LN_EPS,
                    op0=mybir.AluOpType.mult, op1=mybir.AluOpType.add)
                # rstd = var^-0.5 via exp(-0.5*ln(var)): Ln/Exp/Copy/Square
                # share one activation table set (no ATL thrash, unlike Sqrt)
                lnv = ep.tile([128, EGRP], F32, tag="lnv")
                nc.scalar.activation(
                    out=lnv[:, 0:nbk], in_=vv,
                    func=mybir.ActivationFunctionType.Ln)
                rstd = ep.tile([128, EGRP], F32, tag="rstd")
                nc.scalar.activation(
                    out=rstd[:, 0:nbk], in_=lnv[:, 0:nbk],
                    func=mybir.ActivationFunctionType.Exp, scale=-0.5)
                ot = ep.tile([128, EGRP, D], BF16, tag="ot")
                nc.gpsimd.tensor_tensor(
                    out=ot[:, 0:nbk, :], in0=cts,
                    in1=rstd[:, 0:nbk].rearrange("p (b o) -> p b o", o=1)
                        .to_broadcast([128, nbk, D]),
                    op=mybir.AluOpType.mult)
                nc.vector.tensor_tensor(
                    out=ot[:, 0:nbk, :], in0=ot[:, 0:nbk, :],
                    in1=gam_t[:].rearrange("p (o d) -> p o d", o=1)
                        .to_broadcast([128, nbk, D]),
                    op=mybir.AluOpType.mult)
                nc.vector.tensor_tensor(
                    out=ot[:, 0:nbk, :], in0=ot[:, 0:nbk, :],
                    in1=xpb_g[:, 0:nbk, :], op=mybir.AluOpType.add)
                nc.sync.dma_start(
                    out=out[:].rearrange("(p a) d -> p a d", p=128)
                        [:, b0:b0 + nbk, :],
                    in_=ot[:, 0:nbk, :])

            # ---- phase 2
            segs_by_chunk = [[] for _ in range(CHN)]
            for sg in segs:
                segs_by_chunk[sg[0]].append(sg)

            def windows(csegs, cap=256):
                """Split segments into <=cap-tile windows."""
                wins, cur, tn = [], [], 0
                for sg in csegs:
                    if cur and tn + sg[1] * sg[3] > cap:
                        wins.append(cur)
                        cur, tn = [], 0
                    cur.append(sg)
                    tn += sg[1] * sg[3]
                if cur:
                    wins.append(cur)
                return wins

            # chunks 0..2 with interleaved next-chunk table build
            for c in range(CHN - 1):
                csegs = segs_by_chunk[c]
                nseg = len(csegs)
                si = 0
                for win in windows(csegs):
                    t0 = win[0][4]
                    tn = win[-1][4] + win[-1][1] * win[-1][3] - t0
                    load_stream(t0, tn)
                    for cwin in windows(win, CWIN):
                        si += len(cwin)
                        emit_kv_slabs(int(slab_cum[c + 1])
                                      + (CH_SLABS[c + 1] * si) // nseg)
                        if si >= nseg - 1 and si - len(cwin) < nseg - 1:
                            q_gather(c + 1)
                        do_cwindow(c, cwin)

            # phase-1 pools done; free PSUM banks for the epilogue
            p1p_cm.__exit__(None, None, None)
            p1_cm.__exit__(None, None, None)
            ep_cm = tc.tile_pool(name="ep", bufs=1)
            ep = ep_cm.__enter__()
            epp_cm = tc.tile_pool(name="epps", bufs=2, space="PSUM")
            epp = epp_cm.__enter__()

            # chunk 3: combine folded into psum; epilogue per 14-block group
            cbt = []
            for ci in range(CHN - 1):
                cxi = mp.tile([128, NB // 16], I16, tag=f"cbt{ci}")
                nc.scalar.dma_start(
                    out=cxi[:],
                    in_=cbixp[:, ci * (NB // 16):(ci + 1) * (NB // 16)])
                cbt.append(cxi)
            next_grp = 0
            for win in windows(segs_by_chunk[CHN - 1]):
                t0 = win[0][4]
                tn = win[-1][4] + win[-1][1] * win[-1][3] - t0
                load_stream(t0, tn)
                for cwin in windows(win, CWIN):
                    do_cwindow(CHN - 1, cwin, cbt=cbt)
                    done_b = cwin[-1][2] + cwin[-1][3]
                    while next_grp + EGRP <= done_b:
                        epilogue_group(ep, epp, next_grp, EGRP)
                        next_grp += EGRP
            while next_grp < NBLK:
                nbk = min(EGRP, NBLK - next_grp)
                epilogue_group(ep, epp, next_grp, nbk)
                next_grp += nbk

            ep_cm.__exit__(None, None, None)
            epp_cm.__exit__(None, None, None)
            bp_cm.__exit__(None, None, None)
            wp_cm.__exit__(None, None, None)
            mp_cm.__exit__(None, None, None)
            gp_cm.__exit__(None, None, None)
    return nc


def kernel(x, edge_index, edge_weights, Wq, bq, Wk, bk, Wv, bv, Wo, bo,
           gamma, beta):
    x = np.asarray(x, np.float32)
    edge_weights = np.asarray(edge_weights, np.float32)
    origins = np.asarray(edge_index[0], np.int64)
    dests = np.asarray(edge_index[1], np.int64)

    struct, per_core, deg = _build_structure(origins, dests)
    nc = _build_graph(struct)
    nc.finalize()

    perm_t = _slab_perm(NT)
    xpad = np.zeros((NT, D), np.float32)
    xpad[:N] = x
    xT = np.empty((D + 1, NT), np.float32)
    xT[:D] = xpad[perm_t].T
    xT[D] = 1.0
    xT = xT.astype(BF16_NP)

    vperm = (np.arange(H)[None, :] * HD + np.arange(HD)[:, None]).ravel()
    wkv = np.zeros((D + 1, 2 * D), np.float32)
    wkv[:D, :D] = np.asarray(Wk, np.float32).T
    wkv[:D, D:] = np.asarray(Wv, np.float32).T[:, vperm]
    wkv[D, :D] = np.asarray(bk, np.float32)
    wkv[D, D:] = np.asarray(bv, np.float32)[vperm]
    wq = np.zeros((D + 1, D), np.float32)
    wq[:D] = np.asarray(Wq, np.float32).T
    wq[D] = np.asarray(bq, np.float32)
    wot1 = np.ascontiguousarray(np.asarray(Wo, np.float32).T[vperm, :])
    wot = np.zeros((2 * D, 2 * D), np.float32)     # block-diagonal pair form
    wot[:D, :D] = wot1
    wot[D:, D:] = wot1
    bo = np.asarray(bo, np.float32)
    boc = np.tile((bo - bo.mean())[None, :], (128, 1))
    gam_t = np.tile(np.asarray(gamma, np.float32)[None, :], (128, 1))
    idn = np.eye(128, dtype=np.float32)
    beta = np.asarray(beta, np.float32)
    perm_q = _slab_perm(NB)

    in_maps = []
    outs_meta = []
    for ci in range(NCORES):
        data = _per_core_arrays(struct, per_core[ci], deg[ci], edge_weights)
        xo = np.zeros((NB, D), np.float32)
        xo[:NOWN] = x[ci * NOWN:(ci + 1) * NOWN]
        xq_c = np.empty((D + 1, NB), np.float32)
        xq_c[:D] = xo[perm_q].T
        xq_c[D] = 1.0
        order3 = data["order3"]
        xpb_c = (xo[order3] + beta[None, :]).reshape(NBLK, 128, D) \
            .transpose(1, 0, 2).reshape(NB, D)
        in_maps.append({
            "xT": xT, "xq": xq_c.astype(BF16_NP),
            "wkv": wkv.astype(BF16_NP), "wq": wq.astype(BF16_NP),
            "wot": wot.astype(BF16_NP), "boc": boc.astype(BF16_NP),
            "gam": gam_t.astype(BF16_NP), "idnp": idn.astype(BF16_NP),
            "xpb": xpb_c.astype(BF16_NP), "npadp": data["npad"],
            "kvx": data["kvx"], "ewp": data["ew"],
            "qix": data["qix"], "cbix": data["cbix"],
        })
        outs_meta.append(order3)

    global LAST_SIM_NS
    if SIMULATE_COST:
        from concourse import bass_interp
        sim = bass_interp.CoreSim(nc, no_exec=True, publish_trace=False)
        sim.event_loop()
        LAST_SIM_NS = int(sim.time)

    res = run_bass_kernel_spmd(nc, in_maps, core_ids=list(range(NCORES)),
                               trace=TRACE)
    global LAST_RESULT
    LAST_RESULT = res
    outs = []
    for ci in range(NCORES):
        o = np.asarray(res.results[ci]["out"]).astype(np.float32)
        o = o.reshape(128, NBLK, D).transpose(1, 0, 2).reshape(NB, D)
        inv = np.empty(NB, np.int64)
        inv[outs_meta[ci]] = np.arange(NB)
        outs.append(o[inv[:NOWN]])
    return np.concatenate(outs, axis=0)


TRACE = False
SIMULATE_COST = False
LAST_RESULT = None
LAST_SIM_NS = None


# revision 75
# speedup vs baseline: 1.0174x; 1.0056x over previous
"""Trainium2 Bass kernel for BaseDependentAttentionLayer (GNN message passing).

v3 design (8 NeuronCores, SPMD), structured-slot layout:
  - Edges sharded by origin core. Within a core, each of 4 dest-chunks gets its
    OWN degree-sorted origin permutation: chunk-c slot (block b, partition p)
    holds one origin; tile t of block b holds the t-th chunk-c edge of each
    origin in the block (blocks padded to a uniform per-block degree).
  - Consequences: q is a per-partition broadcast from an SBUF table (no
    per-edge q gather); the scatter-reduction matmul uses a CONSTANT identity
    lhsT (no per-edge one-hot gather). Only ONE 256B gather per edge (k|v).
  - Chunk sizes are uneven ([8,16,17,15] slabs): a small chunk 0 shortens the
    prologue (table build) critical path; a smaller chunk 3 shortens the tail.
  - Per-chunk partials (vals|z) drain to DRAM rows [z f32 | vals bf16] 256B;
    chunk 3 drains straight into the SBUF accumulator, and the combine
    (3 per-node gathers + adds) plus the whole epilogue run interleaved with
    the chunk-3 pass per 14-block group.
  - Softmax pad slots hit a zero k|v row with ew=0 so they add exactly 1.0 to
    z; a host-computed npad tile subtracts them in the epilogue.
"""

import sys

sys.path.insert(0, "/opt/trn_rl_repo")

import numpy as np
import ml_dtypes

import concourse.bass as bass
import concourse.bacc as bacc
import concourse.mybir as mybir
from concourse.tile import TileContext
from concourse.bass_utils import run_bass_kernel_spmd

N = 100000
E = 1600000
D = 64
H = 4
HD = 16
NCORES = 8
NOWN = 12500            # nodes owned per core
NBLK = 98               # 128-node blocks per core (final order)
NB = NBLK * 128         # 12544 padded own nodes
SLAB = 1792
CH_SLABS = [8, 16, 17, 15]
CH_ROWS = [s * SLAB for s in CH_SLABS]
CH_R0 = np.concatenate([[0], np.cumsum(CH_ROWS)])   # len 5
CHN = 4
NT = int(CH_R0[-1])     # 100352
PZROW = NB              # zero row index within each partial table
SEG_TILES = 56          # max tiles per equal-degree segment
CWIN = 64               # compute-window tiles (merged elementwise ops)
EGRP = 14               # epilogue block-group size
LN_EPS = 1e-5

F32 = mybir.dt.float32
BF16 = mybir.dt.bfloat16
U32 = mybir.dt.uint32
I16 = mybir.dt.int16
BF16_NP = ml_dtypes.bfloat16

# engine knobs (tuned against the CoreSim cost model)
WS_ON_POOL = True
TREE1_ON_POOL = True
CTB_ON_POOL = False


def _wrap_idx(vals):
    """SWDGE index layout: [16, n/16] wrapped, replicated to 128 partitions."""
    assert len(vals) % 16 == 0
    w = vals.reshape(-1, 16).T.astype(np.int16)
    return np.tile(w, (8, 1))


def _slab_perm(nrows):
    """Column permutation making table-write DMAs contiguous (baseline)."""
    assert nrows % SLAB == 0
    j = np.arange(nrows)
    s, r = j // SLAB, j % SLAB
    a, p = r // 128, r % 128
    return s * SLAB + p * (SLAB // 128) + a


def _build_structure(origins, dests):
    """Common (cross-core max) per-chunk block-degree structure + per-core
    degree/sort data."""
    owner = origins // NOWN
    per_core = []
    for c in range(NCORES):
        m = owner == c
        o = (origins[m] - c * NOWN).astype(np.int32)
        d = dests[m].astype(np.int32)
        w_idx = np.nonzero(m)[0]
        per_core.append((o, d, w_idx))

    deg = np.zeros((NCORES, CHN, NB), np.int64)
    for ci, (o, d, w_idx) in enumerate(per_core):
        ch = np.searchsorted(CH_R0, d, side="right") - 1
        for c in range(CHN):
            deg[ci, c] = np.bincount(o[ch == c], minlength=NB)

    sdeg = -np.sort(-deg, axis=2)            # per-core sorted degree curves
    common = sdeg.max(axis=0)                # [CHN, NB] common slot degrees

    nblocks = []
    block_deg = []
    for c in range(CHN):
        if c < CHN - 1:
            nz = int((common[c] > 0).sum())
            nb = max((nz + 127) // 128, 1)
        else:
            nb = NBLK
        bd = common[c].reshape(NBLK, 128)[:nb, 0].copy()   # block max = first
        if c == CHN - 1:
            bd = np.maximum(bd, 1)           # every final block drains
        nblocks.append(nb)
        block_deg.append(bd.astype(np.int64))

    # segments: runs of equal block degree, capped at SEG_TILES tiles.
    # chunk-3 segments must not straddle EGRP boundaries (epilogue groups).
    segs = []          # (chunk, d, b0, nb, tile_off)
    t_off = 0
    for c in range(CHN):
        bd = block_deg[c]
        b = 0
        while b < nblocks[c]:
            dd = int(bd[b])
            e = b
            while e < nblocks[c] and bd[e] == dd:
                e += 1
            if c == CHN - 1:
                e = min(e, (b // EGRP + 1) * EGRP)
            maxnb = max(SEG_TILES // max(dd, 1), 1)
            nb = min(e - b, maxnb)
            segs.append((c, dd, b, nb, t_off))
            t_off += nb * dd
            b += nb
    return {
        "segs": segs,
        "S_tiles": t_off,
        "nblocks": nblocks,
        "block_deg": block_deg,
    }, per_core, deg


def _per_core_arrays(struct, core_raw, core_deg, edge_weights):
    """Per-core streams: kv idx, ew, q idx, combine idx, npad, orders."""
    o, d, w_idx = core_raw
    ch = np.searchsorted(CH_R0, d, side="right") - 1
    segs = struct["segs"]
    S_tiles = struct["S_tiles"]
    nblocks = struct["nblocks"]
    scale = HD ** -0.5

    orders = []            # slot -> node
    slot_of = []           # node -> slot
    npad = np.zeros(NB, np.float64)
    kvi = np.zeros(S_tiles * 128, np.int32)
    ew4 = np.zeros((S_tiles * 128, H), np.float32)

    for (cc, dd, b0, nb, t_off) in segs:
        kvi[t_off * 128:(t_off + nb * dd) * 128] = CH_ROWS[cc]   # zero row

    for c in range(CHN):
        degc = core_deg[c]
        order = np.argsort(-degc, kind="stable").astype(np.int32)
        inv = np.empty(NB, np.int32)
        inv[order] = np.arange(NB, dtype=np.int32)
        orders.append(order)
        slot_of.append(inv)

        m = ch == c
        oc, dc, wc = o[m], d[m], w_idx[m]
        eorder = np.argsort(inv[oc], kind="stable")
        oc, dc, wc = oc[eorder], dc[eorder], wc[eorder]
        sdeg = degc[order]
        estart = np.zeros(NB + 1, np.int64)
        np.cumsum(sdeg, out=estart[1:])
        ewc = edge_weights[wc] * scale

        for (cc, dd, b0, nb, t_off) in segs:
            if cc != c:
                continue
            for br in range(nb):
                b = b0 + br
                s0 = b * 128
                base = (t_off + br * dd) * 128
                degs = sdeg[s0:s0 + 128]
                nodes = order[s0:s0 + 128]
                npad[nodes] += dd - degs
                for t in range(dd):
                    p = np.nonzero(degs > t)[0]
                    if len(p) == 0:
                        continue
                    ei = estart[s0 + p] + t
                    pos = base + t * 128 + p
                    kvi[pos] = dc[ei] - CH_R0[c]
                    ew4[pos] = ewc[ei]

    ew_t = np.ascontiguousarray(
        ew4.reshape(S_tiles, 128, H).transpose(1, 0, 2)).astype(BF16_NP)
    kvx = _wrap_idx(kvi.astype(np.int16))

    qparts = []
    for c in range(CHN):
        qparts.append(orders[c][:nblocks[c] * 128])
    qix = _wrap_idx(np.concatenate(qparts).astype(np.int16))

    cparts = []
    order3 = orders[CHN - 1]
    for c in range(CHN - 1):
        lim = nblocks[c] * 128
        sc = slot_of[c][order3]
        sc = np.where(sc < lim, sc, PZROW)
        cparts.append(sc)
    cbix = _wrap_idx(np.concatenate(cparts).astype(np.int16))

    npad_t = (npad[order3].astype(np.float32) - 1e-16) \
        .reshape(NBLK, 128).T.copy()

    return {
        "kvx": kvx, "ew": ew_t, "qix": qix, "cbix": cbix,
        "npad": npad_t, "order3": order3,
    }


def _build_graph(struct):
    nc = bacc.Bacc()
    segs = struct["segs"]
    S_tiles = struct["S_tiles"]
    nblocks = struct["nblocks"]

    QCOLS = sum(nblocks) * 8
    CBCOLS = 3 * (NB // 16)

    xT = nc.declare_dram_parameter("xT", [D + 1, NT], BF16, isOutput=False)
    xq = nc.declare_dram_parameter("xq", [D + 1, NB], BF16, isOutput=False)
    wkv = nc.declare_dram_parameter("wkv", [D + 1, 2 * D], BF16,
                                    isOutput=False)
    wq = nc.declare_dram_parameter("wq", [D + 1, D], BF16, isOutput=False)
    wot = nc.declare_dram_parameter("wot", [2 * D, 2 * D + 2], BF16,
                                    isOutput=False)
    boc = nc.declare_dram_parameter("boc", [128, D], BF16, isOutput=False)
    gam = nc.declare_dram_parameter("gam", [128, D], BF16, isOutput=False)
    idnp = nc.declare_dram_parameter("idnp", [128, 128], BF16, isOutput=False)
    xpb = nc.declare_dram_parameter("xpb", [NB, D], BF16, isOutput=False)
    npadp = nc.declare_dram_parameter("npadp", [128, NBLK], F32,
                                      isOutput=False)
    kvxp = nc.declare_dram_parameter("kvx", [128, S_tiles * 8], I16,
                                     isOutput=False)
    ewp = nc.declare_dram_parameter("ewp", [128, S_tiles, H], BF16,
                                    isOutput=False)
    qixp = nc.declare_dram_parameter("qix", [128, QCOLS], I16, isOutput=False)
    cbixp = nc.declare_dram_parameter("cbix", [128, CBCOLS], I16,
                                      isOutput=False)
    out = nc.declare_dram_parameter("out", [NB, D], BF16, isOutput=True)

    kv_tab = [nc.dram_tensor(f"kv_tab{c}", [CH_ROWS[c] + 1, D], U32)
              for c in range(CHN)]
    q_tab = nc.dram_tensor("q_tab", [NB, D], U32)
    partial = [nc.dram_tensor(f"partial{c}", [NB + 1, D], U32)
               for c in range(CHN - 1)]

    qix_off = np.zeros(CHN + 1, np.int64)
    np.cumsum([nblocks[c] * 8 for c in range(CHN)], out=qix_off[1:])
    slab_cum = np.concatenate([[0], np.cumsum(CH_SLABS)])

    with TileContext(nc) as tc:
        with tc.tile_pool(name="const", bufs=1) as cp:
            wkv_t = cp.tile([D + 1, 2 * D], BF16)
            nc.sync.dma_start(out=wkv_t[:], in_=wkv[:])
            wq_t = cp.tile([D + 1, D], BF16)
            nc.sync.dma_start(out=wq_t[:], in_=wq[:])
            wot_t = cp.tile([2 * D, 2 * D + 2], BF16)
            nc.sync.dma_start(out=wot_t[:], in_=wot[:])
            boc_t = cp.tile([128, D], BF16)
            nc.sync.dma_start(out=boc_t[:], in_=boc[:])
            gam_t = cp.tile([128, D], BF16)
            nc.sync.dma_start(out=gam_t[:], in_=gam[:])
            idn_t = cp.tile([128, 128], BF16)
            nc.sync.dma_start(out=idn_t[:], in_=idnp[:])
            npad_t = cp.tile([128, NBLK], F32)
            nc.sync.dma_start(out=npad_t[:], in_=npadp[:])
            acc = cp.tile([128, NBLK, D + H], F32)
            zrow_t = cp.tile([128, D], U32)
            nc.vector.memset(zrow_t[:].bitcast(F32), 0.0)
            for c in range(CHN):
                nc.scalar.dma_start(
                    out=kv_tab[c][CH_ROWS[c]:CH_ROWS[c] + 1, :],
                    in_=zrow_t[0:1, :])
            for c in range(CHN - 1):
                nc.scalar.dma_start(out=partial[c][PZROW:PZROW + 1, :],
                                    in_=zrow_t[0:1, :])

            q_sb = cp.tile([128, 2, NBLK, D], BF16)

            # preload the one activation table set covering Exp/Ln/Copy/Square
            from concourse.hw_specs import get_activation_tables
            _tabs = list(get_activation_tables(nc.m.arch).items())
            _nlx = [i for i, (n, f) in enumerate(_tabs)
                    if n == "natural_log_exp_and_others"][0]
            _atl = mybir.InstLoadActFuncSet(
                name=nc.get_next_instruction_name(), ins=[], outs=[],
                act_func_set_id=_nlx)
            _atl.engine = mybir.EngineType.Activation
            nc.scalar.add_instruction(_atl)

            gp_cm = tc.tile_pool(name="gat", bufs=3)
            gp = gp_cm.__enter__()
            mp_cm = tc.tile_pool(name="met", bufs=2)
            mp = mp_cm.__enter__()
            wp_cm = tc.tile_pool(name="wrk", bufs=2)
            wp = wp_cm.__enter__()
            bp_cm = tc.tile_pool(name="bps", bufs=4, space="PSUM")
            bp = bp_cm.__enter__()
            p1_cm = tc.tile_pool(name="p1sb", bufs=2)
            p1 = p1_cm.__enter__()
            p1p_cm = tc.tile_pool(name="p1ps", bufs=2, space="PSUM")
            p1p = p1p_cm.__enter__()

            slab_no = [0]
            NPRO = 10                 # prologue slabs: q(7) + kv0(5) overlap

            def qkv_slab(src, col0, wt, wcols, tab, row0):
                """One 1792-col slab -> table rows (baseline pattern)."""
                sn = slab_no[0]
                slab_no[0] += 1
                xs = p1.tile([D + 1, SLAB], BF16, tag="xs")
                ((nc.scalar if sn % 2 == 0 else nc.sync)
                 if sn >= 15 else nc.scalar).dma_start(
                    out=xs[:], in_=src[:, col0:col0 + SLAB])
                sb = p1.tile([128, SLAB // 128, wcols], BF16, tag="sb")
                gt = 1024 // wcols
                for g0 in range(0, SLAB // 128, gt):
                    gn = min(gt, SLAB // 128 - g0)
                    ps = p1p.tile([128, 1024], F32, tag="ps")
                    for j in range(gn):
                        jj = g0 + j
                        nc.tensor.matmul(
                            out=ps[:, j * wcols:(j + 1) * wcols],
                            lhsT=xs[:, jj * 128:(jj + 1) * 128],
                            rhs=wt[:], start=True, stop=True)
                    dst = sb[:, g0:g0 + gn, :].rearrange("p a d -> p (a d)")
                    # NOTE: GPSIMD cannot read PSUM on real HW (verifier)
                    if sn < NPRO + 2:
                        eng = nc.vector
                    else:
                        eng = (nc.scalar, nc.scalar, nc.vector)[sn % 3]
                    if eng is nc.scalar:
                        eng.copy(dst, ps[:, 0:gn * wcols])
                    else:
                        eng.tensor_copy(dst, ps[:, 0:gn * wcols])
                nc.sync.dma_start(
                    out=tab[row0:row0 + SLAB, 0:wcols // 2]
                        .bitcast(BF16)
                        .rearrange("(p a) d -> p a d", p=128),
                    in_=sb[:])

            kv_emitted = 0

            def emit_kv_slabs(upto):
                nonlocal kv_emitted
                upto = min(upto, int(slab_cum[-1]))
                while kv_emitted < upto:
                    s = kv_emitted
                    c = int(np.searchsorted(slab_cum, s, side="right")) - 1
                    sl = s - int(slab_cum[c])
                    qkv_slab(xT, int(CH_R0[c]) + sl * SLAB, wkv_t, 2 * D,
                             kv_tab[c], sl * SLAB)
                    kv_emitted += 1

            # prologue: interleave q slabs with kv chunk-0 slabs
            for s in range(NB // SLAB):                  # 7 q slabs
                qkv_slab(xq, s * SLAB, wq_t, D, q_tab, s * SLAB)
                if s % 2 == 1:
                    emit_kv_slabs(kv_emitted + 1)
            emit_kv_slabs(CH_SLABS[0])

            def q_gather(c):
                """Gather chunk-c q rows into q_sb[:, c % 2] (in pieces)."""
                nbk = nblocks[c]
                for h0 in range(0, nbk, SEG_TILES):
                    hn = min(SEG_TILES, nbk - h0)
                    qxi = mp.tile([128, SEG_TILES * 8], I16, tag="qxi")
                    nc.scalar.dma_start(
                        out=qxi[:, 0:hn * 8],
                        in_=qixp[:, qix_off[c] + h0 * 8:
                                 qix_off[c] + (h0 + hn) * 8])
                    qg = gp.tile([128, SEG_TILES, D], U32, tag="kvg")
                    nc.gpsimd.dma_gather(
                        out_ap=qg[:, 0:hn, :], in_ap=q_tab[:],
                        idxs_ap=qxi[:, 0:hn * 8],
                        num_idxs=hn * 128, num_idxs_reg=hn * 128,
                        elem_size=D, single_packet=False)
                    nc.vector.tensor_copy(
                        q_sb[:, c % 2, h0:h0 + hn, :],
                        qg[:, 0:hn, :].bitcast(BF16)[:, :, 0:D])

            q_gather(0)

            # per-chunk batched idx/ew loads; segments slice these tiles
            stream = {"kxi": None, "ew": None, "t0": 0, "tn": 0}

            def load_stream(t0, tn):
                stream["t0"], stream["tn"] = t0, tn
                kxi = mp.tile([128, 256 * 8], I16, tag="kxi")
                nc.sync.dma_start(
                    out=kxi[:, 0:tn * 8],
                    in_=kvxp[:, t0 * 8:(t0 + tn) * 8])
                ew_t = mp.tile([128, 256, H], BF16, tag="ew")
                nc.sync.dma_start(
                    out=ew_t[:, 0:tn, :], in_=ewp[:, t0:t0 + tn, :])
                stream["kxi"], stream["ew"] = kxi, ew_t

            def do_cwindow(c, cwin, cbt=None):
                """One compute window: a run of segments. One gather + one
                merged elementwise chain; qk and the matmul reduction go
                per segment (the q broadcast AP needs uniform degree)."""
                tw0 = cwin[0][4]
                W = cwin[-1][4] + cwin[-1][1] * cwin[-1][3] - tw0
                r0 = tw0 - stream["t0"]
                assert r0 >= 0 and r0 + W <= stream["tn"] and W <= CWIN
                kvg = gp.tile([128, CWIN, D], U32, tag="kvg")
                nc.gpsimd.dma_gather(
                    out_ap=kvg[:, 0:W, :], in_ap=kv_tab[c][:],
                    idxs_ap=stream["kxi"][:, r0 * 8:(r0 + W) * 8],
                    num_idxs=W * 128, num_idxs_reg=W * 128,
                    elem_size=D, single_packet=False)
                ew_t = stream["ew"][:, r0:r0 + W, :]

                kvb = kvg[:, 0:W, :].bitcast(BF16)    # [128, W, 128]
                qk = wp.tile([128, CWIN, D], BF16, tag="qk")
                for (cc, dd, b0, nb, t_off) in cwin:
                    s0 = t_off - tw0
                    T = nb * dd
                    qc = q_sb[:, c % 2, b0:b0 + nb, :] \
                        .rearrange("p b (o d) -> p b o d", o=1) \
                        .to_broadcast([128, nb, dd, D])
                    nc.vector.tensor_tensor(
                        out=qk[:, s0:s0 + T, :]
                            .rearrange("p (b t) d -> p b t d", b=nb),
                        in0=kvb[:, s0:s0 + T, 0:D]
                            .rearrange("p (b t) d -> p b t d", b=nb),
                        in1=qc, op=mybir.AluOpType.mult)

                qk4 = qk[:, 0:W, :].rearrange("p t (h d) -> p t h d", h=H)
                s1 = wp.tile([128, CWIN, 32], BF16, tag="s1")
                s14 = s1[:, 0:W, :].rearrange("p t (h d) -> p t h d", h=H)
                eng_t1 = nc.gpsimd if (TREE1_ON_POOL and c < CHN - 1) \
                    else nc.vector
                eng_t1.tensor_tensor(
                    out=s14, in0=qk4[:, :, :, 0:8], in1=qk4[:, :, :, 8:16],
                    op=mybir.AluOpType.add)
                s2 = wp.tile([128, CWIN, 16], BF16, tag="s2")
                s24 = s2[:, 0:W, :].rearrange("p t (h d) -> p t h d", h=H)
                nc.vector.tensor_tensor(
                    out=s24, in0=s14[:, :, :, 0:4], in1=s14[:, :, :, 4:8],
                    op=mybir.AluOpType.add)
                s3 = wp.tile([128, CWIN, 8], BF16, tag="s3")
                s34 = s3[:, 0:W, :].rearrange("p t (h d) -> p t h d", h=H)
                nc.vector.tensor_tensor(
                    out=s34, in0=s24[:, :, :, 0:2], in1=s24[:, :, :, 2:4],
                    op=mybir.AluOpType.add)
                sc = wp.tile([128, CWIN, H], F32, tag="sc")
                sc4 = sc[:, 0:W, :].rearrange("p t (h o) -> p t h o", h=H)
                (nc.gpsimd if c < CHN - 1 else nc.vector).tensor_tensor(
                    out=sc4, in0=s34[:, :, :, 0:1], in1=s34[:, :, :, 1:2],
                    op=mybir.AluOpType.add)
                ws = wp.tile([128, CWIN, H], F32, tag="ws")
                eng_ws = nc.gpsimd if WS_ON_POOL else nc.vector
                eng_ws.tensor_tensor(
                    out=ws[:, 0:W, :], in0=sc[:, 0:W, :],
                    in1=ew_t, op=mybir.AluOpType.mult)
                wv = wp.tile([128, CWIN, D + H], BF16, tag="wv")
                nc.scalar.activation(
                    out=wv[:, 0:W, D:D + H], in_=ws[:, 0:W, :],
                    func=mybir.ActivationFunctionType.Exp)
                eng_ctb = nc.gpsimd if (CTB_ON_POOL and c == 0) \
                    else nc.vector
                eng_ctb.tensor_tensor(
                    out=wv[:, 0:W, 0:D]
                        .rearrange("p t (e h) -> p t e h", h=H),
                    in0=kvb[:, :, D:2 * D]
                        .rearrange("p t (e h) -> p t e h", h=H),
                    in1=wv[:, 0:W, D:D + H]
                        .rearrange("p t (o h) -> p t o h", o=1)
                        .to_broadcast([128, W, HD, H]),
                    op=mybir.AluOpType.mult)

                for (cc, dd, b0, nb, t_off) in cwin:
                    s0 = t_off - tw0
                    for g0 in range(0, nb, 7):
                        gn = min(7, nb - g0)
                        ncmb = 0 if cbt is None else CHN - 1
                        gcbs = []
                        for ci in range(ncmb):
                            gcb = gp.tile([128, 7, D], U32, tag=f"gcb{ci}")
                            nc.gpsimd.dma_gather(
                                out_ap=gcb[:, 0:gn, :], in_ap=partial[ci][:],
                                idxs_ap=cbt[ci][:, (b0 + g0) * 8:
                                                (b0 + g0 + gn) * 8],
                                num_idxs=gn * 128, num_idxs_reg=gn * 128,
                                elem_size=D, single_packet=False)
                            gcbs.append(gcb)
                        psum = bp.tile([128, 7, D + H], F32, tag="bps")
                        for br in range(gn):
                            for t in range(dd):
                                nc.tensor.matmul(
                                    out=psum[:, br, :], lhsT=idn_t[:],
                                    rhs=wv[:, s0 + (g0 + br) * dd + t, :],
                                    start=(br == 0 and t == 0),
                                    stop=(ncmb == 0 and br == gn - 1
                                          and t == dd - 1))
                        for ci in range(ncmb):
                            gv = gcbs[ci][:].bitcast(BF16)
                            for br in range(gn):
                                nc.tensor.matmul(
                                    out=psum[:, br, :], lhsT=idn_t[:],
                                    rhs=gv[:, br, 0:D + H],
                                    start=False,
                                    stop=(ci == ncmb - 1 and br == gn - 1))
                        if c < CHN - 1:
                            stage = wp.tile([128, 7, 2 * D], BF16, tag="stg")
                            if (b0 + g0) % 2 == 0:
                                nc.scalar.copy(stage[:, 0:gn, 0:D + H],
                                               psum[:, 0:gn, :])
                            else:
                                nc.vector.tensor_copy(
                                    stage[:, 0:gn, 0:D + H],
                                    psum[:, 0:gn, :])
                            nc.sync.dma_start(
                                out=partial[c][(b0 + g0) * 128:
                                               (b0 + g0 + gn) * 128, 0:34]
                                    .rearrange("(a p) d -> p a d", p=128),
                                in_=stage[:, 0:gn, 0:68].bitcast(U32))
                        else:
                            nc.scalar.copy(
                                acc[:, b0 + g0:b0 + g0 + gn, :],
                                psum[:, 0:gn, :])

            def epilogue_group(ep, epp, b0, nbk):
                accs = acc[:, b0:b0 + nbk, :]
                zr = ep.tile([128, EGRP, H], F32, tag="zr")
                nc.vector.tensor_tensor(
                    out=zr[:, 0:nbk, :], in0=accs[:, :, D:D + H],
                    in1=npad_t[:, b0:b0 + nbk]
                        .rearrange("p (b o) -> p b o", o=1)
                        .to_broadcast([128, nbk, H]),
                    op=mybir.AluOpType.subtract)
                # empty/fake rows have z == npad exactly; keep zr finite
                nc.vector.tensor_scalar(
                    out=zr[:, 0:nbk, :], in0=zr[:, 0:nbk, :],
                    scalar1=1e-16, scalar2=None, op0=mybir.AluOpType.max)
                nc.vector.reciprocal(zr[:, 0:nbk, :], zr[:, 0:nbk, :])
                vals = ep.tile([128, EGRP, D], BF16, tag="vals")
                nc.gpsimd.tensor_tensor(
                    out=vals[:, 0:nbk, :]
                        .rearrange("p b (e h) -> p b e h", h=H),
                    in0=accs[:, :, 0:D]
                        .rearrange("p b (e h) -> p b e h", h=H),
                    in1=zr[:, 0:nbk, :]
                        .rearrange("p b (o h) -> p b o h", o=1)
                        .to_broadcast([128, nbk, HD, H]),
                    op=mybir.AluOpType.mult)
                po_sg = ep.tile([128, EGRP, D + 1], BF16, tag="posg")
                npair = (nbk + 1) // 2
                for p0 in range(0, npair, 3):
                    pn = min(3, npair - p0)
                    po = epp.tile([128, 512], F32, tag="po")
                    for pi in range(pn):
                        g = (p0 + pi) * 2
                        pt = epp.tile([128, 128], BF16, tag="pt")
                        nc.tensor.transpose(
                            out=pt[:],
                            in_=vals[:, g:g + 2, :]
                                .rearrange("p a d -> p (a d)"),
                            identity=idn_t[:])
                        vT = ep.tile([128, 128], BF16, tag="vT")
                        nc.scalar.copy(vT[:], pt[:])
                        nc.tensor.matmul(
                            out=po[:, pi * 130:pi * 130 + 130],
                            lhsT=vT[:], rhs=wot_t[:, 0:130],
                            start=True, stop=True)
                    nc.scalar.copy(
                        po_sg[:, p0 * 2:p0 * 2 + pn * 2, :]
                            .rearrange("p a d -> p (a d)"),
                        po[:, 0:pn * 130])
                nmu = po_sg[:, :, D:D + 1]      # -mean via the Wo mu column
                xpb_g = ep.tile([128, EGRP, D], BF16, tag="xpbg")
                nc.scalar.dma_start(
                    out=xpb_g[:, 0:nbk, :],
                    in_=xpb[:].rearrange("(p a) d -> p a d", p=128)
                        [:, b0:b0 + nbk, :])
                ct_g = ep.tile([128, EGRP, D], BF16, tag="ctg")
                cts = ct_g[:, 0:nbk, :]
                nc.gpsimd.tensor_tensor(
                    out=cts, in0=po_sg[:, 0:nbk, 0:D],
                    in1=nmu[:, 0:nbk, :]
                        .to_broadcast([128, nbk, D]),
                    op=mybir.AluOpType.add)
                nc.gpsimd.tensor_tensor(
                    out=cts, in0=cts,
                    in1=boc_t[:].rearrange("p (o d) -> p o d", o=1)
                        .to_broadcast([128, nbk, D]),
                    op=mybir.AluOpType.add)
                sq = ep.tile([128, EGRP, D], F32, tag="sq")
                nc.scalar.square(sq[:, 0:nbk, :], cts)
                vv_t = ep.tile([128, EGRP], F32, tag="vv")
                vv = vv_t[:, 0:nbk]
                nc.vector.tensor_reduce(
                    out=vv, in_=sq[:, 0:nbk, :],
                    axis=mybir.AxisListType.X, op=mybir.AluOpType.add)
                nc.vector.tensor_scalar(
                    out=vv, in0=vv, scalar1=1.0 / D, scalar2=LN_EPS,
                    op0=mybir.AluOpType.mult, op1=mybir.AluOpType.add)
                # rstd = var^-0.5 via exp(-0.5*ln(var)): Ln/Exp/Copy/Square
                # share one activation table set (no ATL thrash, unlike Sqrt)
                lnv = ep.tile([128, EGRP], F32, tag="lnv")
                nc.scalar.activation(
                    out=lnv[:, 0:nbk], in_=vv,
                    func=mybir.ActivationFunctionType.Ln)
                rstd = ep.tile([128, EGRP], F32, tag="rstd")
                nc.scalar.activation(
                    out=rstd[:, 0:nbk], in_=lnv[:, 0:nbk],
                    func=mybir.ActivationFunctionType.Exp, scale=-0.5)
                ot = ep.tile([128, EGRP, D], BF16, tag="ot")
                nc.gpsimd.tensor_tensor(
                    out=ot[:, 0:nbk, :], in0=cts,
                    in1=rstd[:, 0:nbk].rearrange("p (b o) -> p b o", o=1)
                        .to_broadcast([128, nbk, D]),
                    op=mybir.AluOpType.mult)
                nc.vector.tensor_tensor(
                    out=ot[:, 0:nbk, :], in0=ot[:, 0:nbk, :],
                    in1=gam_t[:].rearrange("p (o d) -> p o d", o=1)
                        .to_broadcast([128, nbk, D]),
                    op=mybir.AluOpType.mult)
                nc.vector.tensor_tensor(
                    out=ot[:, 0:nbk, :], in0=ot[:, 0:nbk, :],
                    in1=xpb_g[:, 0:nbk, :], op=mybir.AluOpType.add)
                nc.sync.dma_start(
                    out=out[:].rearrange("(p a) d -> p a d", p=128)
                        [:, b0:b0 + nbk, :],
                    in_=ot[:, 0:nbk, :])

            # ---- phase 2
            segs_by_chunk = [[] for _ in range(CHN)]
            for sg in segs:
                segs_by_chunk[sg[0]].append(sg)

            def windows(csegs, cap=256):
                """Split segments into <=cap-tile windows."""
                wins, cur, tn = [], [], 0
                for sg in csegs:
                    if cur and tn + sg[1] * sg[3] > cap:
                        wins.append(cur)
                        cur, tn = [], 0
                    cur.append(sg)
                    tn += sg[1] * sg[3]
                if cur:
                    wins.append(cur)
                return wins

            # chunks 0..2 with interleaved next-chunk table build
            for c in range(CHN - 1):
                csegs = segs_by_chunk[c]
                nseg = len(csegs)
                si = 0
                for win in windows(csegs):
                    t0 = win[0][4]
                    tn = win[-1][4] + win[-1][1] * win[-1][3] - t0
                    load_stream(t0, tn)
                    for cwin in windows(win, CWIN):
                        si += len(cwin)
                        emit_kv_slabs(int(slab_cum[c + 1])
                                      + (CH_SLABS[c + 1] * si) // nseg)
                        if si >= nseg - 1 and si - len(cwin) < nseg - 1:
                            q_gather(c + 1)
                        do_cwindow(c, cwin)

            # phase-1 pools done; free PSUM banks for the epilogue
            p1p_cm.__exit__(None, None, None)
            p1_cm.__exit__(None, None, None)
            ep_cm = tc.tile_pool(name="ep", bufs=1)
            ep = ep_cm.__enter__()
            epp_cm = tc.tile_pool(name="epps", bufs=2, space="PSUM")
            epp = epp_cm.__enter__()

            # chunk 3: combine folded into psum; epilogue per 14-block group
            cbt = []
            for ci in range(CHN - 1):
                cxi = mp.tile([128, NB // 16], I16, tag=f"cbt{ci}")
                nc.scalar.dma_start(
                    out=cxi[:],
                    in_=cbixp[:, ci * (NB // 16):(ci + 1) * (NB // 16)])
                cbt.append(cxi)
            next_grp = 0
            for win in windows(segs_by_chunk[CHN - 1]):
                t0 = win[0][4]
                tn = win[-1][4] + win[-1][1] * win[-1][3] - t0
                load_stream(t0, tn)
                for cwin in windows(win, CWIN):
                    do_cwindow(CHN - 1, cwin, cbt=cbt)
                    done_b = cwin[-1][2] + cwin[-1][3]
                    while next_grp + EGRP <= done_b:
                        epilogue_group(ep, epp, next_grp, EGRP)
                        next_grp += EGRP
            while next_grp < NBLK:
                nbk = min(EGRP, NBLK - next_grp)
                epilogue_group(ep, epp, next_grp, nbk)
                next_grp += nbk

            ep_cm.__exit__(None, None, None)
            epp_cm.__exit__(None, None, None)
            bp_cm.__exit__(None, None, None)
            wp_cm.__exit__(None, None, None)
            mp_cm.__exit__(None, None, None)
            gp_cm.__exit__(None, None, None)
    return nc


def kernel(x, edge_index, edge_weights, Wq, bq, Wk, bk, Wv, bv, Wo, bo,
           gamma, beta):
    x = np.asarray(x, np.float32)
    edge_weights = np.asarray(edge_weights, np.float32)
    origins = np.asarray(edge_index[0], np.int64)
    dests = np.asarray(edge_index[1], np.int64)

    struct, per_core, deg = _build_structure(origins, dests)
    nc = _build_graph(struct)
    nc.finalize()

    perm_t = _slab_perm(NT)
    xpad = np.zeros((NT, D), np.float32)
    xpad[:N] = x
    xT = np.empty((D + 1, NT), np.float32)
    xT[:D] = xpad[perm_t].T
    xT[D] = 1.0
    xT = xT.astype(BF16_NP)

    vperm = (np.arange(H)[None, :] * HD + np.arange(HD)[:, None]).ravel()
    wkv = np.zeros((D + 1, 2 * D), np.float32)
    wkv[:D, :D] = np.asarray(Wk, np.float32).T
    wkv[:D, D:] = np.asarray(Wv, np.float32).T[:, vperm]
    wkv[D, :D] = np.asarray(bk, np.float32)
    wkv[D, D:] = np.asarray(bv, np.float32)[vperm]
    wq = np.zeros((D + 1, D), np.float32)
    wq[:D] = np.asarray(Wq, np.float32).T
    wq[D] = np.asarray(bq, np.float32)
    wot1 = np.ascontiguousarray(np.asarray(Wo, np.float32).T[vperm, :])
    wot = np.zeros((2 * D, 2 * D + 2), np.float32)  # blkdiag + -mean columns
    wot[:D, :D] = wot1
    wot[:D, D] = -wot1.mean(axis=1)
    wot[D:, D + 1:2 * D + 1] = wot1
    wot[D:, 2 * D + 1] = -wot1.mean(axis=1)
    bo = np.asarray(bo, np.float32)
    boc = np.tile((bo - bo.mean())[None, :], (128, 1))
    gam_t = np.tile(np.asarray(gamma, np.float32)[None, :], (128, 1))
    idn = np.eye(128, dtype=np.float32)
    beta = np.asarray(beta, np.float32)
    perm_q = _slab_perm(NB)

    in_maps = []
    outs_meta = []
    for ci in range(NCORES):
        data = _per_core_arrays(struct, per_core[ci], deg[ci], edge_weights)
        xo = np.zeros((NB, D), np.float32)
        xo[:NOWN] = x[ci * NOWN:(ci + 1) * NOWN]
        xq_c = np.empty((D + 1, NB), np.float32)
        xq_c[:D] = xo[perm_q].T
        xq_c[D] = 1.0
        order3 = data["order3"]
        xpb_c = (xo[order3] + beta[None, :]).reshape(NBLK, 128, D) \
            .transpose(1, 0, 2).reshape(NB, D)
        in_maps.append({
            "xT": xT, "xq": xq_c.astype(BF16_NP),
            "wkv": wkv.astype(BF16_NP), "wq": wq.astype(BF16_NP),
            "wot": wot.astype(BF16_NP), "boc": boc.astype(BF16_NP),
            "gam": gam_t.astype(BF16_NP), "idnp": idn.astype(BF16_NP),
            "xpb": xpb_c.astype(BF16_NP), "npadp": data["npad"],
            "kvx": data["kvx"], "ewp": data["ew"],
            "qix": data["qix"], "cbix": data["cbix"],
        })
        outs_meta.append(order3)

    global LAST_SIM_NS
    if SIMULATE_COST:
        from concourse import bass_interp
        sim = bass_interp.CoreSim(nc, no_exec=True, publish_trace=False)
        sim.event_loop()
        LAST_SIM_NS = int(sim.time)

    res = run_bass_kernel_spmd(nc, in_maps, core_ids=list(range(NCORES)),
                               trace=TRACE)
    global LAST_RESULT
    LAST_RESULT = res
    outs = []
    for ci in range(NCORES):
        o = np.asarray(res.results[ci]["out"]).astype(np.float32)
        o = o.reshape(128, NBLK, D).transpose(1, 0, 2).reshape(NB, D)
        inv = np.empty(NB, np.int64)
        inv[outs_meta[ci]] = np.arange(NB)
        outs.append(o[inv[:NOWN]])
    return np.concatenate(outs, axis=0)


TRACE = False
SIMULATE_COST = False
LAST_RESULT = None
LAST_SIM_NS = None


# revision 80
# speedup vs baseline: 1.0256x; 1.0081x over previous
"""Trainium2 Bass kernel for BaseDependentAttentionLayer (GNN message passing).

v3 design (8 NeuronCores, SPMD), structured-slot layout:
  - Edges sharded by origin core. Within a core, each of 4 dest-chunks gets its
    OWN degree-sorted origin permutation: chunk-c slot (block b, partition p)
    holds one origin; tile t of block b holds the t-th chunk-c edge of each
    origin in the block (blocks padded to a uniform per-block degree).
  - Consequences: q is a per-partition broadcast from an SBUF table (no
    per-edge q gather); the scatter-reduction matmul uses a CONSTANT identity
    lhsT (no per-edge one-hot gather). Only ONE 256B gather per edge (k|v).
  - Chunk sizes are uneven ([8,16,17,15] slabs): a small chunk 0 shortens the
    prologue (table build) critical path; a smaller chunk 3 shortens the tail.
  - Per-chunk partials (vals|z) drain to DRAM rows [z f32 | vals bf16] 256B;
    chunk 3 drains straight into the SBUF accumulator, and the combine
    (3 per-node gathers + adds) plus the whole epilogue run interleaved with
    the chunk-3 pass per 14-block group.
  - Softmax pad slots hit a zero k|v row with ew=0 so they add exactly 1.0 to
    z; a host-computed npad tile subtracts them in the epilogue.
"""

import sys

sys.path.insert(0, "/opt/trn_rl_repo")

import numpy as np
import ml_dtypes

import concourse.bass as bass
import concourse.bacc as bacc
import concourse.mybir as mybir
from concourse.tile import TileContext
from concourse.bass_utils import run_bass_kernel_spmd

N = 100000
E = 1600000
D = 64
H = 4
HD = 16
NCORES = 8
NOWN = 12500            # nodes owned per core
NBLK = 98               # 128-node blocks per core (final order)
NB = NBLK * 128         # 12544 padded own nodes
SLAB = 1792
CH_SLABS = [8, 16, 17, 15]
CH_ROWS = [s * SLAB for s in CH_SLABS]
CH_R0 = np.concatenate([[0], np.cumsum(CH_ROWS)])   # len 5
CHN = 4
NT = int(CH_R0[-1])     # 100352
PZROW = NB              # zero row index within each partial table
SEG_TILES = 56          # max tiles per equal-degree segment
CWIN = 64               # compute-window tiles (merged elementwise ops)
EGRP = 14               # epilogue block-group size
LN_EPS = 1e-5

F32 = mybir.dt.float32
BF16 = mybir.dt.bfloat16
U32 = mybir.dt.uint32
I16 = mybir.dt.int16
BF16_NP = ml_dtypes.bfloat16

# engine knobs (tuned against the CoreSim cost model)
WS_ON_POOL = True
TREE1_ON_POOL = True
CTB_ON_POOL = False


def _wrap_idx(vals):
    """SWDGE index layout: [16, n/16] wrapped, replicated to 128 partitions."""
    assert len(vals) % 16 == 0
    w = vals.reshape(-1, 16).T.astype(np.int16)
    return np.tile(w, (8, 1))


def _slab_perm(nrows):
    """Column permutation making table-write DMAs contiguous (baseline)."""
    assert nrows % SLAB == 0
    j = np.arange(nrows)
    s, r = j // SLAB, j % SLAB
    a, p = r // 128, r % 128
    return s * SLAB + p * (SLAB // 128) + a


def _build_structure(origins, dests):
    """Common (cross-core max) per-chunk block-degree structure + per-core
    degree/sort data."""
    owner = origins // NOWN
    per_core = []
    for c in range(NCORES):
        m = owner == c
        o = (origins[m] - c * NOWN).astype(np.int32)
        d = dests[m].astype(np.int32)
        w_idx = np.nonzero(m)[0]
        per_core.append((o, d, w_idx))

    deg = np.zeros((NCORES, CHN, NB), np.int64)
    for ci, (o, d, w_idx) in enumerate(per_core):
        ch = np.searchsorted(CH_R0, d, side="right") - 1
        for c in range(CHN):
            deg[ci, c] = np.bincount(o[ch == c], minlength=NB)

    sdeg = -np.sort(-deg, axis=2)            # per-core sorted degree curves
    common = sdeg.max(axis=0)                # [CHN, NB] common slot degrees

    nblocks = []
    block_deg = []
    for c in range(CHN):
        if c < CHN - 1:
            nz = int((common[c] > 0).sum())
            nb = max((nz + 127) // 128, 1)
        else:
            nb = NBLK
        bd = common[c].reshape(NBLK, 128)[:nb, 0].copy()   # block max = first
        if c == CHN - 1:
            bd = np.maximum(bd, 1)           # every final block drains
        nblocks.append(nb)
        block_deg.append(bd.astype(np.int64))

    # segments: runs of equal block degree, capped at SEG_TILES tiles.
    # chunk-3 segments must not straddle EGRP boundaries (epilogue groups).
    segs = []          # (chunk, d, b0, nb, tile_off)
    t_off = 0
    for c in range(CHN):
        bd = block_deg[c]
        b = 0
        while b < nblocks[c]:
            dd = int(bd[b])
            e = b
            while e < nblocks[c] and bd[e] == dd:
                e += 1
            if c == CHN - 1:
                e = min(e, (b // EGRP + 1) * EGRP)
            maxnb = max(SEG_TILES // max(dd, 1), 1)
            nb = min(e - b, maxnb)
            segs.append((c, dd, b, nb, t_off))
            t_off += nb * dd
            b += nb
    return {
        "segs": segs,
        "S_tiles": t_off,
        "nblocks": nblocks,
        "block_deg": block_deg,
    }, per_core, deg


def _per_core_arrays(struct, core_raw, core_deg, edge_weights):
    """Per-core streams: kv idx, ew, q idx, combine idx, npad, orders."""
    o, d, w_idx = core_raw
    ch = np.searchsorted(CH_R0, d, side="right") - 1
    segs = struct["segs"]
    S_tiles = struct["S_tiles"]
    nblocks = struct["nblocks"]
    scale = HD ** -0.5

    orders = []            # slot -> node
    slot_of = []           # node -> slot
    npad = np.zeros(NB, np.float64)
    kvi = np.zeros(S_tiles * 128, np.int32)
    ew4 = np.zeros((S_tiles * 128, H), np.float32)

    for (cc, dd, b0, nb, t_off) in segs:
        kvi[t_off * 128:(t_off + nb * dd) * 128] = CH_ROWS[cc]   # zero row

    for c in range(CHN):
        degc = core_deg[c]
        order = np.argsort(-degc, kind="stable").astype(np.int32)
        inv = np.empty(NB, np.int32)
        inv[order] = np.arange(NB, dtype=np.int32)
        orders.append(order)
        slot_of.append(inv)

        m = ch == c
        oc, dc, wc = o[m], d[m], w_idx[m]
        eorder = np.argsort(inv[oc], kind="stable")
        oc, dc, wc = oc[eorder], dc[eorder], wc[eorder]
        sdeg = degc[order]
        estart = np.zeros(NB + 1, np.int64)
        np.cumsum(sdeg, out=estart[1:])
        ewc = edge_weights[wc] * scale

        for (cc, dd, b0, nb, t_off) in segs:
            if cc != c:
                continue
            for br in range(nb):
                b = b0 + br
                s0 = b * 128
                base = (t_off + br * dd) * 128
                degs = sdeg[s0:s0 + 128]
                nodes = order[s0:s0 + 128]
                npad[nodes] += dd - degs
                for t in range(dd):
                    p = np.nonzero(degs > t)[0]
                    if len(p) == 0:
                        continue
                    ei = estart[s0 + p] + t
                    pos = base + t * 128 + p
                    kvi[pos] = dc[ei] - CH_R0[c]
                    ew4[pos] = ewc[ei]

    ew_t = np.ascontiguousarray(
        ew4.reshape(S_tiles, 128, H).transpose(1, 0, 2)).astype(BF16_NP)
    kvx = _wrap_idx(kvi.astype(np.int16))

    qparts = []
    for c in range(CHN):
        qparts.append(orders[c][:nblocks[c] * 128])
    qix = _wrap_idx(np.concatenate(qparts).astype(np.int16))

    cparts = []
    order3 = orders[CHN - 1]
    for c in range(CHN - 1):
        lim = nblocks[c] * 128
        sc = slot_of[c][order3]
        sc = np.where(sc < lim, sc, PZROW)
        cparts.append(sc)
    cbix = _wrap_idx(np.concatenate(cparts).astype(np.int16))

    npad_t = (npad[order3].astype(np.float32) - 1e-16) \
        .reshape(NBLK, 128).T.copy()

    return {
        "kvx": kvx, "ew": ew_t, "qix": qix, "cbix": cbix,
        "npad": npad_t, "order3": order3,
    }


def _build_graph(struct):
    nc = bacc.Bacc()
    segs = struct["segs"]
    S_tiles = struct["S_tiles"]
    nblocks = struct["nblocks"]

    QCOLS = sum(nblocks) * 8
    CBCOLS = 3 * (NB // 16)

    xT = nc.declare_dram_parameter("xT", [D + 1, NT], BF16, isOutput=False)
    xq = nc.declare_dram_parameter("xq", [D + 1, NB], BF16, isOutput=False)
    wkv = nc.declare_dram_parameter("wkv", [D + 1, 2 * D], BF16,
                                    isOutput=False)
    wq = nc.declare_dram_parameter("wq", [D + 1, D], BF16, isOutput=False)
    wot = nc.declare_dram_parameter("wot", [2 * D, 2 * D + 2], BF16,
                                    isOutput=False)
    boc = nc.declare_dram_parameter("boc", [128, D], BF16, isOutput=False)
    gam = nc.declare_dram_parameter("gam", [128, D], BF16, isOutput=False)
    idnp = nc.declare_dram_parameter("idnp", [128, 128], BF16, isOutput=False)
    xpb = nc.declare_dram_parameter("xpb", [NB, D], BF16, isOutput=False)
    npadp = nc.declare_dram_parameter("npadp", [128, NBLK], F32,
                                      isOutput=False)
    kvxp = nc.declare_dram_parameter("kvx", [128, S_tiles * 8], I16,
                                     isOutput=False)
    ewp = nc.declare_dram_parameter("ewp", [128, S_tiles, H], BF16,
                                    isOutput=False)
    qixp = nc.declare_dram_parameter("qix", [128, QCOLS], I16, isOutput=False)
    cbixp = nc.declare_dram_parameter("cbix", [128, CBCOLS], I16,
                                      isOutput=False)
    out = nc.declare_dram_parameter("out", [NB, D], BF16, isOutput=True)

    kv_tab = [nc.dram_tensor(f"kv_tab{c}", [CH_ROWS[c] + 1, D], U32)
              for c in range(CHN)]
    q_tab = nc.dram_tensor("q_tab", [NB, D], U32)
    partial = [nc.dram_tensor(f"partial{c}", [NB + 1, D], U32)
               for c in range(CHN - 1)]

    qix_off = np.zeros(CHN + 1, np.int64)
    np.cumsum([nblocks[c] * 8 for c in range(CHN)], out=qix_off[1:])
    slab_cum = np.concatenate([[0], np.cumsum(CH_SLABS)])

    with TileContext(nc) as tc:
        with tc.tile_pool(name="const", bufs=1) as cp:
            wkv_t = cp.tile([D + 1, 2 * D], BF16)
            nc.sync.dma_start(out=wkv_t[:], in_=wkv[:])
            wq_t = cp.tile([D + 1, D], BF16)
            nc.sync.dma_start(out=wq_t[:], in_=wq[:])
            wot_t = cp.tile([2 * D, 2 * D + 2], BF16)
            nc.sync.dma_start(out=wot_t[:], in_=wot[:])
            boc_t = cp.tile([128, D], BF16)
            nc.sync.dma_start(out=boc_t[:], in_=boc[:])
            gam_t = cp.tile([128, D], BF16)
            nc.sync.dma_start(out=gam_t[:], in_=gam[:])
            idn_t = cp.tile([128, 128], BF16)
            nc.sync.dma_start(out=idn_t[:], in_=idnp[:])
            npad_t = cp.tile([128, NBLK], F32)
            nc.sync.dma_start(out=npad_t[:], in_=npadp[:])
            acc = cp.tile([128, NBLK, D + H], F32)
            zrow_t = cp.tile([128, D], U32)
            nc.vector.memset(zrow_t[:].bitcast(F32), 0.0)
            for c in range(CHN):
                nc.scalar.dma_start(
                    out=kv_tab[c][CH_ROWS[c]:CH_ROWS[c] + 1, :],
                    in_=zrow_t[0:1, :])
            for c in range(CHN - 1):
                nc.scalar.dma_start(out=partial[c][PZROW:PZROW + 1, :],
                                    in_=zrow_t[0:1, :])

            q_sb = cp.tile([128, 2, NBLK, D], BF16)

            # preload the one activation table set covering Exp/Ln/Copy/Square
            from concourse.hw_specs import get_activation_tables
            _tabs = list(get_activation_tables(nc.m.arch).items())
            _nlx = [i for i, (n, f) in enumerate(_tabs)
                    if n == "natural_log_exp_and_others"][0]
            _atl = mybir.InstLoadActFuncSet(
                name=nc.get_next_instruction_name(), ins=[], outs=[],
                act_func_set_id=_nlx)
            _atl.engine = mybir.EngineType.Activation
            nc.scalar.add_instruction(_atl)

            gp_cm = tc.tile_pool(name="gat", bufs=3)
            gp = gp_cm.__enter__()
            mp_cm = tc.tile_pool(name="met", bufs=2)
            mp = mp_cm.__enter__()
            wp_cm = tc.tile_pool(name="wrk", bufs=2)
            wp = wp_cm.__enter__()
            bp_cm = tc.tile_pool(name="bps", bufs=4, space="PSUM")
            bp = bp_cm.__enter__()
            p1_cm = tc.tile_pool(name="p1sb", bufs=2)
            p1 = p1_cm.__enter__()
            p1p_cm = tc.tile_pool(name="p1ps", bufs=2, space="PSUM")
            p1p = p1p_cm.__enter__()

            slab_no = [0]
            NPRO = 10                 # prologue slabs: q(7) + kv0(5) overlap

            def qkv_slab(src, col0, wt, wcols, tab, row0):
                """One 1792-col slab -> table rows (baseline pattern)."""
                sn = slab_no[0]
                slab_no[0] += 1
                xs = p1.tile([D + 1, SLAB], BF16, tag="xs")
                ((nc.scalar if sn % 2 == 0 else nc.sync)
                 if sn >= 15 else nc.scalar).dma_start(
                    out=xs[:], in_=src[:, col0:col0 + SLAB])
                sb = p1.tile([128, SLAB // 128, wcols], BF16, tag="sb")
                gt = 1024 // wcols
                for g0 in range(0, SLAB // 128, gt):
                    gn = min(gt, SLAB // 128 - g0)
                    ps = p1p.tile([128, 1024], F32, tag="ps")
                    for j in range(gn):
                        jj = g0 + j
                        nc.tensor.matmul(
                            out=ps[:, j * wcols:(j + 1) * wcols],
                            lhsT=xs[:, jj * 128:(jj + 1) * 128],
                            rhs=wt[:], start=True, stop=True)
                    dst = sb[:, g0:g0 + gn, :].rearrange("p a d -> p (a d)")
                    # NOTE: GPSIMD cannot read PSUM on real HW (verifier)
                    if sn < NPRO + 2:
                        eng = nc.vector
                    else:
                        eng = (nc.scalar, nc.scalar, nc.scalar, nc.vector)[sn % 4]
                    if eng is nc.scalar:
                        eng.copy(dst, ps[:, 0:gn * wcols])
                    else:
                        eng.tensor_copy(dst, ps[:, 0:gn * wcols])
                nc.sync.dma_start(
                    out=tab[row0:row0 + SLAB, 0:wcols // 2]
                        .bitcast(BF16)
                        .rearrange("(p a) d -> p a d", p=128),
                    in_=sb[:])

            kv_emitted = 0

            def emit_kv_slabs(upto):
                nonlocal kv_emitted
                upto = min(upto, int(slab_cum[-1]))
                while kv_emitted < upto:
                    s = kv_emitted
                    c = int(np.searchsorted(slab_cum, s, side="right")) - 1
                    sl = s - int(slab_cum[c])
                    qkv_slab(xT, int(CH_R0[c]) + sl * SLAB, wkv_t, 2 * D,
                             kv_tab[c], sl * SLAB)
                    kv_emitted += 1

            # prologue: interleave q slabs with kv chunk-0 slabs
            for s in range(NB // SLAB):                  # 7 q slabs
                qkv_slab(xq, s * SLAB, wq_t, D, q_tab, s * SLAB)
                if s % 2 == 1:
                    emit_kv_slabs(kv_emitted + 1)
            emit_kv_slabs(CH_SLABS[0])

            def q_gather(c):
                """Gather chunk-c q rows into q_sb[:, c % 2] (in pieces)."""
                nbk = nblocks[c]
                for h0 in range(0, nbk, SEG_TILES):
                    hn = min(SEG_TILES, nbk - h0)
                    qxi = mp.tile([128, SEG_TILES * 8], I16, tag="qxi")
                    nc.scalar.dma_start(
                        out=qxi[:, 0:hn * 8],
                        in_=qixp[:, qix_off[c] + h0 * 8:
                                 qix_off[c] + (h0 + hn) * 8])
                    qg = gp.tile([128, SEG_TILES, D], U32, tag="kvg")
                    nc.gpsimd.dma_gather(
                        out_ap=qg[:, 0:hn, :], in_ap=q_tab[:],
                        idxs_ap=qxi[:, 0:hn * 8],
                        num_idxs=hn * 128, num_idxs_reg=hn * 128,
                        elem_size=D, single_packet=False)
                    nc.vector.tensor_copy(
                        q_sb[:, c % 2, h0:h0 + hn, :],
                        qg[:, 0:hn, :].bitcast(BF16)[:, :, 0:D])

            q_gather(0)

            # per-chunk batched idx/ew loads; segments slice these tiles
            stream = {"kxi": None, "ew": None, "t0": 0, "tn": 0}

            def load_stream(t0, tn):
                stream["t0"], stream["tn"] = t0, tn
                kxi = mp.tile([128, 256 * 8], I16, tag="kxi")
                nc.sync.dma_start(
                    out=kxi[:, 0:tn * 8],
                    in_=kvxp[:, t0 * 8:(t0 + tn) * 8])
                ew_t = mp.tile([128, 256, H], BF16, tag="ew")
                nc.sync.dma_start(
                    out=ew_t[:, 0:tn, :], in_=ewp[:, t0:t0 + tn, :])
                stream["kxi"], stream["ew"] = kxi, ew_t

            def do_cwindow(c, cwin, cbt=None):
                """One compute window: a run of segments. One gather + one
                merged elementwise chain; qk and the matmul reduction go
                per segment (the q broadcast AP needs uniform degree)."""
                tw0 = cwin[0][4]
                W = cwin[-1][4] + cwin[-1][1] * cwin[-1][3] - tw0
                r0 = tw0 - stream["t0"]
                assert r0 >= 0 and r0 + W <= stream["tn"] and W <= CWIN
                kvg = gp.tile([128, CWIN, D], U32, tag="kvg")
                nc.gpsimd.dma_gather(
                    out_ap=kvg[:, 0:W, :], in_ap=kv_tab[c][:],
                    idxs_ap=stream["kxi"][:, r0 * 8:(r0 + W) * 8],
                    num_idxs=W * 128, num_idxs_reg=W * 128,
                    elem_size=D, single_packet=False)
                ew_t = stream["ew"][:, r0:r0 + W, :]

                gcb_pre = {}
                if cbt is not None:
                    for (c2, d2, b2, nb2, t2) in cwin:
                        for g0 in range(0, nb2, 7):
                            gn = min(7, nb2 - g0)
                            for ci in range(CHN - 1):
                                gcb = gp.tile([128, 7, D], U32,
                                              tag=f"gcb{ci}")
                                nc.gpsimd.dma_gather(
                                    out_ap=gcb[:, 0:gn, :],
                                    in_ap=partial[ci][:],
                                    idxs_ap=cbt[ci][:, (b2 + g0) * 8:
                                                    (b2 + g0 + gn) * 8],
                                    num_idxs=gn * 128,
                                    num_idxs_reg=gn * 128,
                                    elem_size=D, single_packet=False)
                                gcb_pre[(b2 + g0, ci)] = gcb
                kvb = kvg[:, 0:W, :].bitcast(BF16)    # [128, W, 128]
                qk = wp.tile([128, CWIN, D], BF16, tag="qk")
                for (cc, dd, b0, nb, t_off) in cwin:
                    s0 = t_off - tw0
                    T = nb * dd
                    qc = q_sb[:, c % 2, b0:b0 + nb, :] \
                        .rearrange("p b (o d) -> p b o d", o=1) \
                        .to_broadcast([128, nb, dd, D])
                    nc.vector.tensor_tensor(
                        out=qk[:, s0:s0 + T, :]
                            .rearrange("p (b t) d -> p b t d", b=nb),
                        in0=kvb[:, s0:s0 + T, 0:D]
                            .rearrange("p (b t) d -> p b t d", b=nb),
                        in1=qc, op=mybir.AluOpType.mult)

                qk4 = qk[:, 0:W, :].rearrange("p t (h d) -> p t h d", h=H)
                s1 = wp.tile([128, CWIN, 32], BF16, tag="s1")
                s14 = s1[:, 0:W, :].rearrange("p t (h d) -> p t h d", h=H)
                eng_t1 = nc.gpsimd if (TREE1_ON_POOL and c < CHN - 1) \
                    else nc.vector
                eng_t1.tensor_tensor(
                    out=s14, in0=qk4[:, :, :, 0:8], in1=qk4[:, :, :, 8:16],
                    op=mybir.AluOpType.add)
                s2 = wp.tile([128, CWIN, 16], BF16, tag="s2")
                s24 = s2[:, 0:W, :].rearrange("p t (h d) -> p t h d", h=H)
                nc.vector.tensor_tensor(
                    out=s24, in0=s14[:, :, :, 0:4], in1=s14[:, :, :, 4:8],
                    op=mybir.AluOpType.add)
                s3 = wp.tile([128, CWIN, 8], BF16, tag="s3")
                s34 = s3[:, 0:W, :].rearrange("p t (h d) -> p t h d", h=H)
                (nc.gpsimd if c < CHN - 1 else nc.vector).tensor_tensor(
                    out=s34, in0=s24[:, :, :, 0:2], in1=s24[:, :, :, 2:4],
                    op=mybir.AluOpType.add)
                sc = wp.tile([128, CWIN, H], F32, tag="sc")
                sc4 = sc[:, 0:W, :].rearrange("p t (h o) -> p t h o", h=H)
                (nc.gpsimd if c < CHN - 1 else nc.vector).tensor_tensor(
                    out=sc4, in0=s34[:, :, :, 0:1], in1=s34[:, :, :, 1:2],
                    op=mybir.AluOpType.add)
                ws = wp.tile([128, CWIN, H], F32, tag="ws")
                eng_ws = nc.gpsimd if WS_ON_POOL else nc.vector
                eng_ws.tensor_tensor(
                    out=ws[:, 0:W, :], in0=sc[:, 0:W, :],
                    in1=ew_t, op=mybir.AluOpType.mult)
                wv = wp.tile([128, CWIN, D + H], BF16, tag="wv")
                nc.scalar.activation(
                    out=wv[:, 0:W, D:D + H], in_=ws[:, 0:W, :],
                    func=mybir.ActivationFunctionType.Exp)
                eng_ctb = nc.gpsimd if (CTB_ON_POOL and c == 0) \
                    else nc.vector
                eng_ctb.tensor_tensor(
                    out=wv[:, 0:W, 0:D]
                        .rearrange("p t (e h) -> p t e h", h=H),
                    in0=kvb[:, :, D:2 * D]
                        .rearrange("p t (e h) -> p t e h", h=H),
                    in1=wv[:, 0:W, D:D + H]
                        .rearrange("p t (o h) -> p t o h", o=1)
                        .to_broadcast([128, W, HD, H]),
                    op=mybir.AluOpType.mult)

                for (cc, dd, b0, nb, t_off) in cwin:
                    s0 = t_off - tw0
                    for g0 in range(0, nb, 7):
                        gn = min(7, nb - g0)
                        ncmb = 0 if cbt is None else CHN - 1
                        gcbs = [gcb_pre[(b0 + g0, ci)]
                                for ci in range(ncmb)]
                        psum = bp.tile([128, 7, D + H], F32, tag="bps")
                        for br in range(gn):
                            for t in range(dd):
                                nc.tensor.matmul(
                                    out=psum[:, br, :], lhsT=idn_t[:],
                                    rhs=wv[:, s0 + (g0 + br) * dd + t, :],
                                    start=(br == 0 and t == 0),
                                    stop=(ncmb == 0 and br == gn - 1
                                          and t == dd - 1))
                        for ci in range(ncmb):
                            gv = gcbs[ci][:].bitcast(BF16)
                            for br in range(gn):
                                nc.tensor.matmul(
                                    out=psum[:, br, :], lhsT=idn_t[:],
                                    rhs=gv[:, br, 0:D + H],
                                    start=False,
                                    stop=(ci == ncmb - 1 and br == gn - 1))
                        if c < CHN - 1:
                            stage = wp.tile([128, 7, 2 * D], BF16, tag="stg")
                            if (b0 + g0) % 2 == 0:
                                nc.scalar.copy(stage[:, 0:gn, 0:D + H],
                                               psum[:, 0:gn, :])
                            else:
                                nc.vector.tensor_copy(
                                    stage[:, 0:gn, 0:D + H],
                                    psum[:, 0:gn, :])
                            nc.sync.dma_start(
                                out=partial[c][(b0 + g0) * 128:
                                               (b0 + g0 + gn) * 128, 0:34]
                                    .rearrange("(a p) d -> p a d", p=128),
                                in_=stage[:, 0:gn, 0:68].bitcast(U32))
                        else:
                            nc.scalar.copy(
                                acc[:, b0 + g0:b0 + g0 + gn, :],
                                psum[:, 0:gn, :])

            def epilogue_group(ep, epp, b0, nbk):
                accs = acc[:, b0:b0 + nbk, :]
                zr = ep.tile([128, EGRP, H], F32, tag="zr")
                nc.vector.tensor_tensor(
                    out=zr[:, 0:nbk, :], in0=accs[:, :, D:D + H],
                    in1=npad_t[:, b0:b0 + nbk]
                        .rearrange("p (b o) -> p b o", o=1)
                        .to_broadcast([128, nbk, H]),
                    op=mybir.AluOpType.subtract)
                # empty/fake rows have z == npad exactly; keep zr finite
                nc.vector.tensor_scalar(
                    out=zr[:, 0:nbk, :], in0=zr[:, 0:nbk, :],
                    scalar1=1e-16, scalar2=None, op0=mybir.AluOpType.max)
                nc.vector.reciprocal(zr[:, 0:nbk, :], zr[:, 0:nbk, :])
                vals = ep.tile([128, EGRP, D], BF16, tag="vals")
                nc.gpsimd.tensor_tensor(
                    out=vals[:, 0:nbk, :]
                        .rearrange("p b (e h) -> p b e h", h=H),
                    in0=accs[:, :, 0:D]
                        .rearrange("p b (e h) -> p b e h", h=H),
                    in1=zr[:, 0:nbk, :]
                        .rearrange("p b (o h) -> p b o h", o=1)
                        .to_broadcast([128, nbk, HD, H]),
                    op=mybir.AluOpType.mult)
                po_sg = ep.tile([128, EGRP, D + 1], BF16, tag="posg")
                npair = (nbk + 1) // 2
                for p0 in range(0, npair, 3):
                    pn = min(3, npair - p0)
                    po = epp.tile([128, 512], F32, tag="po")
                    for pi in range(pn):
                        g = (p0 + pi) * 2
                        pt = epp.tile([128, 128], BF16, tag="pt")
                        nc.tensor.transpose(
                            out=pt[:],
                            in_=vals[:, g:g + 2, :]
                                .rearrange("p a d -> p (a d)"),
                            identity=idn_t[:])
                        vT = ep.tile([128, 128], BF16, tag="vT")
                        nc.scalar.copy(vT[:], pt[:])
                        nc.tensor.matmul(
                            out=po[:, pi * 130:pi * 130 + 130],
                            lhsT=vT[:], rhs=wot_t[:, 0:130],
                            start=True, stop=True)
                    nc.scalar.copy(
                        po_sg[:, p0 * 2:p0 * 2 + pn * 2, :]
                            .rearrange("p a d -> p (a d)"),
                        po[:, 0:pn * 130])
                nmu = po_sg[:, :, D:D + 1]      # -mean via the Wo mu column
                xpb_g = ep.tile([128, EGRP, D], BF16, tag="xpbg")
                nc.scalar.dma_start(
                    out=xpb_g[:, 0:nbk, :],
                    in_=xpb[:].rearrange("(p a) d -> p a d", p=128)
                        [:, b0:b0 + nbk, :])
                ct_g = ep.tile([128, EGRP, D], BF16, tag="ctg")
                cts = ct_g[:, 0:nbk, :]
                nc.gpsimd.tensor_tensor(
                    out=cts, in0=po_sg[:, 0:nbk, 0:D],
                    in1=nmu[:, 0:nbk, :]
                        .to_broadcast([128, nbk, D]),
                    op=mybir.AluOpType.add)
                nc.gpsimd.tensor_tensor(
                    out=cts, in0=cts,
                    in1=boc_t[:].rearrange("p (o d) -> p o d", o=1)
                        .to_broadcast([128, nbk, D]),
                    op=mybir.AluOpType.add)
                sq = ep.tile([128, EGRP, D], F32, tag="sq")
                nc.scalar.square(sq[:, 0:nbk, :], cts)
                vv_t = ep.tile([128, EGRP], F32, tag="vv")
                vv = vv_t[:, 0:nbk]
                nc.vector.tensor_reduce(
                    out=vv, in_=sq[:, 0:nbk, :],
                    axis=mybir.AxisListType.X, op=mybir.AluOpType.add)
                nc.vector.tensor_scalar(
                    out=vv, in0=vv, scalar1=1.0 / D, scalar2=LN_EPS,
                    op0=mybir.AluOpType.mult, op1=mybir.AluOpType.add)
                # rstd = var^-0.5 via exp(-0.5*ln(var)): Ln/Exp/Copy/Square
                # share one activation table set (no ATL thrash, unlike Sqrt)
                lnv = ep.tile([128, EGRP], F32, tag="lnv")
                nc.scalar.activation(
                    out=lnv[:, 0:nbk], in_=vv,
                    func=mybir.ActivationFunctionType.Ln)
                rstd = ep.tile([128, EGRP], F32, tag="rstd")
                nc.scalar.activation(
                    out=rstd[:, 0:nbk], in_=lnv[:, 0:nbk],
                    func=mybir.ActivationFunctionType.Exp, scale=-0.5)
                ot = ep.tile([128, EGRP, D], BF16, tag="ot")
                nc.gpsimd.tensor_tensor(
                    out=ot[:, 0:nbk, :], in0=cts,
                    in1=rstd[:, 0:nbk].rearrange("p (b o) -> p b o", o=1)
                        .to_broadcast([128, nbk, D]),
                    op=mybir.AluOpType.mult)
                nc.vector.tensor_tensor(
                    out=ot[:, 0:nbk, :], in0=ot[:, 0:nbk, :],
                    in1=gam_t[:].rearrange("p (o d) -> p o d", o=1)
                        .to_broadcast([128, nbk, D]),
                    op=mybir.AluOpType.mult)
                nc.vector.tensor_tensor(
                    out=ot[:, 0:nbk, :], in0=ot[:, 0:nbk, :],
                    in1=xpb_g[:, 0:nbk, :], op=mybir.AluOpType.add)
                nc.sync.dma_start(
                    out=out[:].rearrange("(p a) d -> p a d", p=128)
                        [:, b0:b0 + nbk, :],
                    in_=ot[:, 0:nbk, :])

            # ---- phase 2
            segs_by_chunk = [[] for _ in range(CHN)]
            for sg in segs:
                segs_by_chunk[sg[0]].append(sg)

            def windows(csegs, cap=256):
                """Split segments into <=cap-tile windows."""
                wins, cur, tn = [], [], 0
                for sg in csegs:
                    if cur and tn + sg[1] * sg[3] > cap:
                        wins.append(cur)
                        cur, tn = [], 0
                    cur.append(sg)
                    tn += sg[1] * sg[3]
                if cur:
                    wins.append(cur)
                return wins

            # chunks 0..2 with interleaved next-chunk table build
            for c in range(CHN - 1):
                csegs = segs_by_chunk[c]
                nseg = len(csegs)
                si = 0
                for win in windows(csegs):
                    t0 = win[0][4]
                    tn = win[-1][4] + win[-1][1] * win[-1][3] - t0
                    load_stream(t0, tn)
                    for cwin in windows(win, CWIN):
                        si += len(cwin)
                        emit_kv_slabs(int(slab_cum[c + 1])
                                      + (CH_SLABS[c + 1] * si) // nseg)
                        if si >= nseg - 1 and si - len(cwin) < nseg - 1:
                            q_gather(c + 1)
                        do_cwindow(c, cwin)

            # phase-1 pools done; free PSUM banks for the epilogue
            p1p_cm.__exit__(None, None, None)
            p1_cm.__exit__(None, None, None)
            ep_cm = tc.tile_pool(name="ep", bufs=1)
            ep = ep_cm.__enter__()
            epp_cm = tc.tile_pool(name="epps", bufs=2, space="PSUM")
            epp = epp_cm.__enter__()

            # chunk 3: combine folded into psum; epilogue per 14-block group
            cbt = []
            for ci in range(CHN - 1):
                cxi = mp.tile([128, NB // 16], I16, tag=f"cbt{ci}")
                nc.scalar.dma_start(
                    out=cxi[:],
                    in_=cbixp[:, ci * (NB // 16):(ci + 1) * (NB // 16)])
                cbt.append(cxi)
            next_grp = 0
            for win in windows(segs_by_chunk[CHN - 1]):
                t0 = win[0][4]
                tn = win[-1][4] + win[-1][1] * win[-1][3] - t0
                load_stream(t0, tn)
                for cwin in windows(win, CWIN):
                    do_cwindow(CHN - 1, cwin, cbt=cbt)
                    done_b = cwin[-1][2] + cwin[-1][3]
                    while next_grp + EGRP <= done_b:
                        epilogue_group(ep, epp, next_grp, EGRP)
                        next_grp += EGRP
            while next_grp < NBLK:
                nbk = min(EGRP, NBLK - next_grp)
                epilogue_group(ep, epp, next_grp, nbk)
                next_grp += nbk

            ep_cm.__exit__(None, None, None)
            epp_cm.__exit__(None, None, None)
            bp_cm.__exit__(None, None, None)
            wp_cm.__exit__(None, None, None)
            mp_cm.__exit__(None, None, None)
            gp_cm.__exit__(None, None, None)
    return nc


def kernel(x, edge_index, edge_weights, Wq, bq, Wk, bk, Wv, bv, Wo, bo,
           gamma, beta):
    x = np.asarray(x, np.float32)
    edge_weights = np.asarray(edge_weights, np.float32)
    origins = np.asarray(edge_index[0], np.int64)
    dests = np.asarray(edge_index[1], np.int64)

    struct, per_core, deg = _build_structure(origins, dests)
    nc = _build_graph(struct)
    nc.finalize()

    perm_t = _slab_perm(NT)
    xpad = np.zeros((NT, D), np.float32)
    xpad[:N] = x
    xT = np.empty((D + 1, NT), np.float32)
    xT[:D] = xpad[perm_t].T
    xT[D] = 1.0
    xT = xT.astype(BF16_NP)

    vperm = (np.arange(H)[None, :] * HD + np.arange(HD)[:, None]).ravel()
    wkv = np.zeros((D + 1, 2 * D), np.float32)
    wkv[:D, :D] = np.asarray(Wk, np.float32).T
    wkv[:D, D:] = np.asarray(Wv, np.float32).T[:, vperm]
    wkv[D, :D] = np.asarray(bk, np.float32)
    wkv[D, D:] = np.asarray(bv, np.float32)[vperm]
    wq = np.zeros((D + 1, D), np.float32)
    wq[:D] = np.asarray(Wq, np.float32).T
    wq[D] = np.asarray(bq, np.float32)
    wot1 = np.ascontiguousarray(np.asarray(Wo, np.float32).T[vperm, :])
    wot = np.zeros((2 * D, 2 * D + 2), np.float32)  # blkdiag + -mean columns
    wot[:D, :D] = wot1
    wot[:D, D] = -wot1.mean(axis=1)
    wot[D:, D + 1:2 * D + 1] = wot1
    wot[D:, 2 * D + 1] = -wot1.mean(axis=1)
    bo = np.asarray(bo, np.float32)
    boc = np.tile((bo - bo.mean())[None, :], (128, 1))
    gam_t = np.tile(np.asarray(gamma, np.float32)[None, :], (128, 1))
    idn = np.eye(128, dtype=np.float32)
    beta = np.asarray(beta, np.float32)
    perm_q = _slab_perm(NB)

    in_maps = []
    outs_meta = []
    for ci in range(NCORES):
        data = _per_core_arrays(struct, per_core[ci], deg[ci], edge_weights)
        xo = np.zeros((NB, D), np.float32)
        xo[:NOWN] = x[ci * NOWN:(ci + 1) * NOWN]
        xq_c = np.empty((D + 1, NB), np.float32)
        xq_c[:D] = xo[perm_q].T
        xq_c[D] = 1.0
        order3 = data["order3"]
        xpb_c = (xo[order3] + beta[None, :]).reshape(NBLK, 128, D) \
            .transpose(1, 0, 2).reshape(NB, D)
        in_maps.append({
            "xT": xT, "xq": xq_c.astype(BF16_NP),
            "wkv": wkv.astype(BF16_NP), "wq": wq.astype(BF16_NP),
            "wot": wot.astype(BF16_NP), "boc": boc.astype(BF16_NP),
            "gam": gam_t.astype(BF16_NP), "idnp": idn.astype(BF16_NP),
            "xpb": xpb_c.astype(BF16_NP), "npadp": data["npad"],
            "kvx": data["kvx"], "ewp": data["ew"],
            "qix": data["qix"], "cbix": data["cbix"],
        })
        outs_meta.append(order3)

    global LAST_SIM_NS
    if SIMULATE_COST:
        from concourse import bass_interp
        sim = bass_interp.CoreSim(nc, no_exec=True, publish_trace=False)
        sim.event_loop()
        LAST_SIM_NS = int(sim.time)

    res = run_bass_kernel_spmd(nc, in_maps, core_ids=list(range(NCORES)),
                               trace=TRACE)
    global LAST_RESULT
    LAST_RESULT = res
    outs = []
    for ci in range(NCORES):
        o = np.asarray(res.results[ci]["out"]).astype(np.float32)
        o = o.reshape(128, NBLK, D).transpose(1, 0, 2).reshape(NB, D)
        inv = np.empty(NB, np.int64)
        inv[outs_meta[ci]] = np.arange(NB)
        outs.append(o[inv[:NOWN]])
    return np.concatenate(outs, axis=0)


TRACE = False
SIMULATE_COST = False
LAST_RESULT = None
LAST_SIM_NS = None


# revision 81
# speedup vs baseline: 1.0430x; 1.0169x over previous
"""Trainium2 Bass kernel for BaseDependentAttentionLayer (GNN message passing).

v3 design (8 NeuronCores, SPMD), structured-slot layout:
  - Edges sharded by origin core. Within a core, each of 4 dest-chunks gets its
    OWN degree-sorted origin permutation: chunk-c slot (block b, partition p)
    holds one origin; tile t of block b holds the t-th chunk-c edge of each
    origin in the block (blocks padded to a uniform per-block degree).
  - Consequences: q is a per-partition broadcast from an SBUF table (no
    per-edge q gather); the scatter-reduction matmul uses a CONSTANT identity
    lhsT (no per-edge one-hot gather). Only ONE 256B gather per edge (k|v).
  - Chunk sizes are uneven ([8,16,17,15] slabs): a small chunk 0 shortens the
    prologue (table build) critical path; a smaller chunk 3 shortens the tail.
  - Per-chunk partials (vals|z) drain to DRAM rows [z f32 | vals bf16] 256B;
    chunk 3 drains straight into the SBUF accumulator, and the combine
    (3 per-node gathers + adds) plus the whole epilogue run interleaved with
    the chunk-3 pass per 14-block group.
  - Softmax pad slots hit a zero k|v row with ew=0 so they add exactly 1.0 to
    z; a host-computed npad tile subtracts them in the epilogue.
"""

import sys

sys.path.insert(0, "/opt/trn_rl_repo")

import numpy as np
import ml_dtypes

import concourse.bass as bass
import concourse.bacc as bacc
import concourse.mybir as mybir
from concourse.tile import TileContext
from concourse.bass_utils import run_bass_kernel_spmd

N = 100000
E = 1600000
D = 64
H = 4
HD = 16
NCORES = 8
NOWN = 12500            # nodes owned per core
NBLK = 98               # 128-node blocks per core (final order)
NB = NBLK * 128         # 12544 padded own nodes
SLAB = 1792
CH_SLABS = [8, 16, 17, 15]
CH_ROWS = [s * SLAB for s in CH_SLABS]
CH_R0 = np.concatenate([[0], np.cumsum(CH_ROWS)])   # len 5
CHN = 4
NT = int(CH_R0[-1])     # 100352
PZROW = NB              # zero row index within each partial table
SEG_TILES = 56          # max tiles per equal-degree segment
CWIN = 64               # compute-window tiles (merged elementwise ops)
EGRP = 14               # epilogue block-group size
LN_EPS = 1e-5

F32 = mybir.dt.float32
BF16 = mybir.dt.bfloat16
U32 = mybir.dt.uint32
I16 = mybir.dt.int16
BF16_NP = ml_dtypes.bfloat16

# engine knobs (tuned against the CoreSim cost model)
WS_ON_POOL = True
TREE1_ON_POOL = True
CTB_ON_POOL = False


def _wrap_idx(vals):
    """SWDGE index layout: [16, n/16] wrapped, replicated to 128 partitions."""
    assert len(vals) % 16 == 0
    w = vals.reshape(-1, 16).T.astype(np.int16)
    return np.tile(w, (8, 1))


def _slab_perm(nrows):
    """Column permutation making table-write DMAs contiguous (baseline)."""
    assert nrows % SLAB == 0
    j = np.arange(nrows)
    s, r = j // SLAB, j % SLAB
    a, p = r // 128, r % 128
    return s * SLAB + p * (SLAB // 128) + a


def _build_structure(origins, dests):
    """Common (cross-core max) per-chunk block-degree structure + per-core
    degree/sort data."""
    owner = origins // NOWN
    per_core = []
    for c in range(NCORES):
        m = owner == c
        o = (origins[m] - c * NOWN).astype(np.int32)
        d = dests[m].astype(np.int32)
        w_idx = np.nonzero(m)[0]
        per_core.append((o, d, w_idx))

    deg = np.zeros((NCORES, CHN, NB), np.int64)
    for ci, (o, d, w_idx) in enumerate(per_core):
        ch = np.searchsorted(CH_R0, d, side="right") - 1
        for c in range(CHN):
            deg[ci, c] = np.bincount(o[ch == c], minlength=NB)

    sdeg = -np.sort(-deg, axis=2)            # per-core sorted degree curves
    common = sdeg.max(axis=0)                # [CHN, NB] common slot degrees

    nblocks = []
    block_deg = []
    for c in range(CHN):
        if c < CHN - 1:
            nz = int((common[c] > 0).sum())
            nb = max((nz + 127) // 128, 1)
        else:
            nb = NBLK
        bd = common[c].reshape(NBLK, 128)[:nb, 0].copy()   # block max = first
        if c == CHN - 1:
            bd = np.maximum(bd, 1)           # every final block drains
        nblocks.append(nb)
        block_deg.append(bd.astype(np.int64))

    # segments: runs of equal block degree, capped at SEG_TILES tiles.
    # chunk-3 segments must not straddle EGRP boundaries (epilogue groups).
    segs = []          # (chunk, d, b0, nb, tile_off)
    t_off = 0
    for c in range(CHN):
        bd = block_deg[c]
        b = 0
        while b < nblocks[c]:
            dd = int(bd[b])
            e = b
            while e < nblocks[c] and bd[e] == dd:
                e += 1
            if c == CHN - 1:
                e = min(e, (b // EGRP + 1) * EGRP)
            maxnb = max(SEG_TILES // max(dd, 1), 1)
            nb = min(e - b, maxnb)
            segs.append((c, dd, b, nb, t_off))
            t_off += nb * dd
            b += nb
    return {
        "segs": segs,
        "S_tiles": t_off,
        "nblocks": nblocks,
        "block_deg": block_deg,
    }, per_core, deg


def _per_core_arrays(struct, core_raw, core_deg, edge_weights):
    """Per-core streams: kv idx, ew, q idx, combine idx, npad, orders."""
    o, d, w_idx = core_raw
    ch = np.searchsorted(CH_R0, d, side="right") - 1
    segs = struct["segs"]
    S_tiles = struct["S_tiles"]
    nblocks = struct["nblocks"]
    scale = HD ** -0.5

    orders = []            # slot -> node
    slot_of = []           # node -> slot
    npad = np.zeros(NB, np.float64)
    kvi = np.zeros(S_tiles * 128, np.int32)
    ew4 = np.zeros((S_tiles * 128, H), np.float32)

    for (cc, dd, b0, nb, t_off) in segs:
        kvi[t_off * 128:(t_off + nb * dd) * 128] = CH_ROWS[cc]   # zero row

    for c in range(CHN):
        degc = core_deg[c]
        order = np.argsort(-degc, kind="stable").astype(np.int32)
        inv = np.empty(NB, np.int32)
        inv[order] = np.arange(NB, dtype=np.int32)
        orders.append(order)
        slot_of.append(inv)

        m = ch == c
        oc, dc, wc = o[m], d[m], w_idx[m]
        eorder = np.argsort(inv[oc], kind="stable")
        oc, dc, wc = oc[eorder], dc[eorder], wc[eorder]
        sdeg = degc[order]
        estart = np.zeros(NB + 1, np.int64)
        np.cumsum(sdeg, out=estart[1:])
        ewc = edge_weights[wc] * scale

        for (cc, dd, b0, nb, t_off) in segs:
            if cc != c:
                continue
            for br in range(nb):
                b = b0 + br
                s0 = b * 128
                base = (t_off + br * dd) * 128
                degs = sdeg[s0:s0 + 128]
                nodes = order[s0:s0 + 128]
                npad[nodes] += dd - degs
                for t in range(dd):
                    p = np.nonzero(degs > t)[0]
                    if len(p) == 0:
                        continue
                    ei = estart[s0 + p] + t
                    pos = base + t * 128 + p
                    kvi[pos] = dc[ei] - CH_R0[c]
                    ew4[pos] = ewc[ei]

    ew_t = np.ascontiguousarray(
        ew4.reshape(S_tiles, 128, H).transpose(1, 0, 2)).astype(BF16_NP)
    kvx = _wrap_idx(kvi.astype(np.int16))

    qparts = []
    for c in range(CHN):
        qparts.append(orders[c][:nblocks[c] * 128])
    qix = _wrap_idx(np.concatenate(qparts).astype(np.int16))

    cparts = []
    order3 = orders[CHN - 1]
    for c in range(CHN - 1):
        lim = nblocks[c] * 128
        sc = slot_of[c][order3]
        sc = np.where(sc < lim, sc, PZROW)
        cparts.append(sc)
    cbix = _wrap_idx(np.concatenate(cparts).astype(np.int16))

    npad_t = (npad[order3].astype(np.float32) - 1e-16) \
        .reshape(NBLK, 128).T.copy()

    return {
        "kvx": kvx, "ew": ew_t, "qix": qix, "cbix": cbix,
        "npad": npad_t, "order3": order3,
    }


def _build_graph(struct):
    nc = bacc.Bacc()
    segs = struct["segs"]
    S_tiles = struct["S_tiles"]
    nblocks = struct["nblocks"]

    QCOLS = sum(nblocks) * 8
    CBCOLS = 3 * (NB // 16)

    xT = nc.declare_dram_parameter("xT", [D + 1, NT], BF16, isOutput=False)
    xq = nc.declare_dram_parameter("xq", [D + 1, NB], BF16, isOutput=False)
    wkv = nc.declare_dram_parameter("wkv", [D + 1, 2 * D], BF16,
                                    isOutput=False)
    wq = nc.declare_dram_parameter("wq", [D + 1, D], BF16, isOutput=False)
    wot = nc.declare_dram_parameter("wot", [2 * D, 2 * D + 2], BF16,
                                    isOutput=False)
    boc = nc.declare_dram_parameter("boc", [128, D], BF16, isOutput=False)
    gam = nc.declare_dram_parameter("gam", [128, D], BF16, isOutput=False)
    idnp = nc.declare_dram_parameter("idnp", [128, 128], BF16, isOutput=False)
    xpb = nc.declare_dram_parameter("xpb", [NB, D], BF16, isOutput=False)
    npadp = nc.declare_dram_parameter("npadp", [128, NBLK], F32,
                                      isOutput=False)
    kvxp = nc.declare_dram_parameter("kvx", [128, S_tiles * 8], I16,
                                     isOutput=False)
    ewp = nc.declare_dram_parameter("ewp", [128, S_tiles, H], BF16,
                                    isOutput=False)
    qixp = nc.declare_dram_parameter("qix", [128, QCOLS], I16, isOutput=False)
    cbixp = nc.declare_dram_parameter("cbix", [128, CBCOLS], I16,
                                      isOutput=False)
    out = nc.declare_dram_parameter("out", [NB, D], BF16, isOutput=True)

    kv_tab = [nc.dram_tensor(f"kv_tab{c}", [CH_ROWS[c] + 1, D], U32)
              for c in range(CHN)]
    q_tab = nc.dram_tensor("q_tab", [NB, D], U32)
    partial = [nc.dram_tensor(f"partial{c}", [NB + 1, D], U32)
               for c in range(CHN - 1)]

    qix_off = np.zeros(CHN + 1, np.int64)
    np.cumsum([nblocks[c] * 8 for c in range(CHN)], out=qix_off[1:])
    slab_cum = np.concatenate([[0], np.cumsum(CH_SLABS)])

    with TileContext(nc) as tc:
        with tc.tile_pool(name="const", bufs=1) as cp:
            wkv_t = cp.tile([D + 1, 2 * D], BF16)
            nc.sync.dma_start(out=wkv_t[:], in_=wkv[:])
            wq_t = cp.tile([D + 1, D], BF16)
            nc.sync.dma_start(out=wq_t[:], in_=wq[:])
            wot_t = cp.tile([2 * D, 2 * D + 2], BF16)
            nc.sync.dma_start(out=wot_t[:], in_=wot[:])
            boc_t = cp.tile([128, D], BF16)
            nc.sync.dma_start(out=boc_t[:], in_=boc[:])
            gam_t = cp.tile([128, D], BF16)
            nc.sync.dma_start(out=gam_t[:], in_=gam[:])
            idn_t = cp.tile([128, 128], BF16)
            nc.sync.dma_start(out=idn_t[:], in_=idnp[:])
            npad_t = cp.tile([128, NBLK], F32)
            nc.sync.dma_start(out=npad_t[:], in_=npadp[:])
            acc = cp.tile([128, NBLK, D + H], F32)
            zrow_t = cp.tile([128, D], U32)
            nc.vector.memset(zrow_t[:].bitcast(F32), 0.0)
            for c in range(CHN):
                nc.scalar.dma_start(
                    out=kv_tab[c][CH_ROWS[c]:CH_ROWS[c] + 1, :],
                    in_=zrow_t[0:1, :])
            for c in range(CHN - 1):
                nc.scalar.dma_start(out=partial[c][PZROW:PZROW + 1, :],
                                    in_=zrow_t[0:1, :])

            q_sb = cp.tile([128, 2, NBLK, D], BF16)

            # preload the one activation table set covering Exp/Ln/Copy/Square
            from concourse.hw_specs import get_activation_tables
            _tabs = list(get_activation_tables(nc.m.arch).items())
            _nlx = [i for i, (n, f) in enumerate(_tabs)
                    if n == "natural_log_exp_and_others"][0]
            _atl = mybir.InstLoadActFuncSet(
                name=nc.get_next_instruction_name(), ins=[], outs=[],
                act_func_set_id=_nlx)
            _atl.engine = mybir.EngineType.Activation
            nc.scalar.add_instruction(_atl)

            gp_cm = tc.tile_pool(name="gat", bufs=3)
            gp = gp_cm.__enter__()
            mp_cm = tc.tile_pool(name="met", bufs=2)
            mp = mp_cm.__enter__()
            wp_cm = tc.tile_pool(name="wrk", bufs=2)
            wp = wp_cm.__enter__()
            bp_cm = tc.tile_pool(name="bps", bufs=4, space="PSUM")
            bp = bp_cm.__enter__()
            p1_cm = tc.tile_pool(name="p1sb", bufs=2)
            p1 = p1_cm.__enter__()
            p1p_cm = tc.tile_pool(name="p1ps", bufs=2, space="PSUM")
            p1p = p1p_cm.__enter__()

            slab_no = [0]
            NPRO = 10                 # prologue slabs: q(7) + kv0(5) overlap

            def qkv_slab(src, col0, wt, wcols, tab, row0):
                """One 1792-col slab -> table rows (baseline pattern)."""
                sn = slab_no[0]
                slab_no[0] += 1
                xs = p1.tile([D + 1, SLAB], BF16, tag="xs")
                ((nc.scalar if sn % 2 == 0 else nc.sync)
                 if sn >= 15 else nc.scalar).dma_start(
                    out=xs[:], in_=src[:, col0:col0 + SLAB])
                sb = p1.tile([128, SLAB // 128, wcols], BF16, tag="sb")
                gt = 1024 // wcols
                for g0 in range(0, SLAB // 128, gt):
                    gn = min(gt, SLAB // 128 - g0)
                    ps = p1p.tile([128, 1024], F32, tag="ps")
                    for j in range(gn):
                        jj = g0 + j
                        nc.tensor.matmul(
                            out=ps[:, j * wcols:(j + 1) * wcols],
                            lhsT=xs[:, jj * 128:(jj + 1) * 128],
                            rhs=wt[:], start=True, stop=True)
                    dst = sb[:, g0:g0 + gn, :].rearrange("p a d -> p (a d)")
                    # NOTE: GPSIMD cannot read PSUM on real HW (verifier)
                    if sn < NPRO + 2:
                        eng = nc.vector
                    else:
                        eng = (nc.scalar, nc.scalar, nc.scalar, nc.vector)[sn % 4]
                    if eng is nc.scalar:
                        eng.copy(dst, ps[:, 0:gn * wcols])
                    else:
                        eng.tensor_copy(dst, ps[:, 0:gn * wcols])
                nc.sync.dma_start(
                    out=tab[row0:row0 + SLAB, 0:wcols // 2]
                        .bitcast(BF16)
                        .rearrange("(p a) d -> p a d", p=128),
                    in_=sb[:])

            kv_emitted = 0

            def emit_kv_slabs(upto):
                nonlocal kv_emitted
                upto = min(upto, int(slab_cum[-1]))
                while kv_emitted < upto:
                    s = kv_emitted
                    c = int(np.searchsorted(slab_cum, s, side="right")) - 1
                    sl = s - int(slab_cum[c])
                    qkv_slab(xT, int(CH_R0[c]) + sl * SLAB, wkv_t, 2 * D,
                             kv_tab[c], sl * SLAB)
                    kv_emitted += 1

            # prologue: interleave q slabs with kv chunk-0 slabs
            for s in range(NB // SLAB):                  # 7 q slabs
                qkv_slab(xq, s * SLAB, wq_t, D, q_tab, s * SLAB)
                if s % 2 == 1:
                    emit_kv_slabs(kv_emitted + 1)
            emit_kv_slabs(CH_SLABS[0])

            def q_gather(c):
                """Gather chunk-c q rows into q_sb[:, c % 2] (in pieces)."""
                nbk = nblocks[c]
                for h0 in range(0, nbk, SEG_TILES):
                    hn = min(SEG_TILES, nbk - h0)
                    qxi = mp.tile([128, SEG_TILES * 8], I16, tag="qxi")
                    nc.scalar.dma_start(
                        out=qxi[:, 0:hn * 8],
                        in_=qixp[:, qix_off[c] + h0 * 8:
                                 qix_off[c] + (h0 + hn) * 8])
                    qg = gp.tile([128, SEG_TILES, D], U32, tag="kvg")
                    nc.gpsimd.dma_gather(
                        out_ap=qg[:, 0:hn, :], in_ap=q_tab[:],
                        idxs_ap=qxi[:, 0:hn * 8],
                        num_idxs=hn * 128, num_idxs_reg=hn * 128,
                        elem_size=D, single_packet=False)
                    nc.vector.tensor_copy(
                        q_sb[:, c % 2, h0:h0 + hn, :],
                        qg[:, 0:hn, :].bitcast(BF16)[:, :, 0:D])

            q_gather(0)

            # per-chunk batched idx/ew loads; segments slice these tiles
            stream = {"kxi": None, "ew": None, "t0": 0, "tn": 0}

            def load_stream(t0, tn):
                stream["t0"], stream["tn"] = t0, tn
                kxi = mp.tile([128, 256 * 8], I16, tag="kxi")
                nc.sync.dma_start(
                    out=kxi[:, 0:tn * 8],
                    in_=kvxp[:, t0 * 8:(t0 + tn) * 8])
                ew_t = mp.tile([128, 256, H], BF16, tag="ew")
                nc.sync.dma_start(
                    out=ew_t[:, 0:tn, :], in_=ewp[:, t0:t0 + tn, :])
                stream["kxi"], stream["ew"] = kxi, ew_t

            def do_cwindow(c, cwin, cbt=None):
                """One compute window: a run of segments. One gather + one
                merged elementwise chain; qk and the matmul reduction go
                per segment (the q broadcast AP needs uniform degree)."""
                tw0 = cwin[0][4]
                W = cwin[-1][4] + cwin[-1][1] * cwin[-1][3] - tw0
                r0 = tw0 - stream["t0"]
                assert r0 >= 0 and r0 + W <= stream["tn"] and W <= CWIN
                kvg = gp.tile([128, CWIN, D], U32, tag="kvg")
                nc.gpsimd.dma_gather(
                    out_ap=kvg[:, 0:W, :], in_ap=kv_tab[c][:],
                    idxs_ap=stream["kxi"][:, r0 * 8:(r0 + W) * 8],
                    num_idxs=W * 128, num_idxs_reg=W * 128,
                    elem_size=D, single_packet=False)
                ew_t = stream["ew"][:, r0:r0 + W, :]

                gcb_pre = {}
                if cbt is not None:
                    for (c2, d2, b2, nb2, t2) in cwin:
                        for g0 in range(0, nb2, 7):
                            gn = min(7, nb2 - g0)
                            for ci in range(CHN - 1):
                                gcb = gp.tile([128, 7, D], U32,
                                              tag=f"gcb{ci}")
                                nc.gpsimd.dma_gather(
                                    out_ap=gcb[:, 0:gn, :],
                                    in_ap=partial[ci][:],
                                    idxs_ap=cbt[ci][:, (b2 + g0) * 8:
                                                    (b2 + g0 + gn) * 8],
                                    num_idxs=gn * 128,
                                    num_idxs_reg=gn * 128,
                                    elem_size=D, single_packet=False)
                                gcb_pre[(b2 + g0, ci)] = gcb
                kvb = kvg[:, 0:W, :].bitcast(BF16)    # [128, W, 128]
                qk = wp.tile([128, CWIN, D], BF16, tag="qk")
                for (cc, dd, b0, nb, t_off) in cwin:
                    s0 = t_off - tw0
                    T = nb * dd
                    qc = q_sb[:, c % 2, b0:b0 + nb, :] \
                        .rearrange("p b (o d) -> p b o d", o=1) \
                        .to_broadcast([128, nb, dd, D])
                    nc.vector.tensor_tensor(
                        out=qk[:, s0:s0 + T, :]
                            .rearrange("p (b t) d -> p b t d", b=nb),
                        in0=kvb[:, s0:s0 + T, 0:D]
                            .rearrange("p (b t) d -> p b t d", b=nb),
                        in1=qc, op=mybir.AluOpType.mult)

                qk4 = qk[:, 0:W, :].rearrange("p t (h d) -> p t h d", h=H)
                s1 = wp.tile([128, CWIN, 32], BF16, tag="s1")
                s14 = s1[:, 0:W, :].rearrange("p t (h d) -> p t h d", h=H)
                eng_t1 = nc.gpsimd if (TREE1_ON_POOL and c < CHN - 1) \
                    else nc.vector
                eng_t1.tensor_tensor(
                    out=s14, in0=qk4[:, :, :, 0:8], in1=qk4[:, :, :, 8:16],
                    op=mybir.AluOpType.add)
                s2 = wp.tile([128, CWIN, 16], BF16, tag="s2")
                s24 = s2[:, 0:W, :].rearrange("p t (h d) -> p t h d", h=H)
                (nc.gpsimd if c < CHN - 1 else nc.vector).tensor_tensor(
                    out=s24, in0=s14[:, :, :, 0:4], in1=s14[:, :, :, 4:8],
                    op=mybir.AluOpType.add)
                s3 = wp.tile([128, CWIN, 8], BF16, tag="s3")
                s34 = s3[:, 0:W, :].rearrange("p t (h d) -> p t h d", h=H)
                (nc.gpsimd if c < CHN - 1 else nc.vector).tensor_tensor(
                    out=s34, in0=s24[:, :, :, 0:2], in1=s24[:, :, :, 2:4],
                    op=mybir.AluOpType.add)
                sc = wp.tile([128, CWIN, H], F32, tag="sc")
                sc4 = sc[:, 0:W, :].rearrange("p t (h o) -> p t h o", h=H)
                (nc.gpsimd if c < CHN - 1 else nc.vector).tensor_tensor(
                    out=sc4, in0=s34[:, :, :, 0:1], in1=s34[:, :, :, 1:2],
                    op=mybir.AluOpType.add)
                ws = wp.tile([128, CWIN, H], F32, tag="ws")
                eng_ws = nc.gpsimd if WS_ON_POOL else nc.vector
                eng_ws.tensor_tensor(
                    out=ws[:, 0:W, :], in0=sc[:, 0:W, :],
                    in1=ew_t, op=mybir.AluOpType.mult)
                wv = wp.tile([128, CWIN, D + H], BF16, tag="wv")
                nc.scalar.activation(
                    out=wv[:, 0:W, D:D + H], in_=ws[:, 0:W, :],
                    func=mybir.ActivationFunctionType.Exp)
                eng_ctb = nc.gpsimd if (CTB_ON_POOL and c == 0) \
                    else nc.vector
                eng_ctb.tensor_tensor(
                    out=wv[:, 0:W, 0:D]
                        .rearrange("p t (e h) -> p t e h", h=H),
                    in0=kvb[:, :, D:2 * D]
                        .rearrange("p t (e h) -> p t e h", h=H),
                    in1=wv[:, 0:W, D:D + H]
                        .rearrange("p t (o h) -> p t o h", o=1)
                        .to_broadcast([128, W, HD, H]),
                    op=mybir.AluOpType.mult)

                for (cc, dd, b0, nb, t_off) in cwin:
                    s0 = t_off - tw0
                    for g0 in range(0, nb, 7):
                        gn = min(7, nb - g0)
                        ncmb = 0 if cbt is None else CHN - 1
                        gcbs = [gcb_pre[(b0 + g0, ci)]
                                for ci in range(ncmb)]
                        psum = bp.tile([128, 7, D + H], F32, tag="bps")
                        for br in range(gn):
                            for t in range(dd):
                                nc.tensor.matmul(
                                    out=psum[:, br, :], lhsT=idn_t[:],
                                    rhs=wv[:, s0 + (g0 + br) * dd + t, :],
                                    start=(br == 0 and t == 0),
                                    stop=(ncmb == 0 and br == gn - 1
                                          and t == dd - 1))
                        for ci in range(ncmb):
                            gv = gcbs[ci][:].bitcast(BF16)
                            for br in range(gn):
                                nc.tensor.matmul(
                                    out=psum[:, br, :], lhsT=idn_t[:],
                                    rhs=gv[:, br, 0:D + H],
                                    start=False,
                                    stop=(ci == ncmb - 1 and br == gn - 1))
                        if c < CHN - 1:
                            stage = wp.tile([128, 7, 2 * D], BF16, tag="stg")
                            if (b0 + g0) % 2 == 0:
                                nc.scalar.copy(stage[:, 0:gn, 0:D + H],
                                               psum[:, 0:gn, :])
                            else:
                                nc.vector.tensor_copy(
                                    stage[:, 0:gn, 0:D + H],
                                    psum[:, 0:gn, :])
                            nc.sync.dma_start(
                                out=partial[c][(b0 + g0) * 128:
                                               (b0 + g0 + gn) * 128, 0:34]
                                    .rearrange("(a p) d -> p a d", p=128),
                                in_=stage[:, 0:gn, 0:68].bitcast(U32))
                        else:
                            nc.scalar.copy(
                                acc[:, b0 + g0:b0 + g0 + gn, :],
                                psum[:, 0:gn, :])

            def epilogue_group(ep, epp, b0, nbk):
                accs = acc[:, b0:b0 + nbk, :]
                zr = ep.tile([128, EGRP, H], F32, tag="zr")
                nc.vector.tensor_tensor(
                    out=zr[:, 0:nbk, :], in0=accs[:, :, D:D + H],
                    in1=npad_t[:, b0:b0 + nbk]
                        .rearrange("p (b o) -> p b o", o=1)
                        .to_broadcast([128, nbk, H]),
                    op=mybir.AluOpType.subtract)
                # empty/fake rows have z == npad exactly; keep zr finite
                nc.vector.tensor_scalar(
                    out=zr[:, 0:nbk, :], in0=zr[:, 0:nbk, :],
                    scalar1=1e-16, scalar2=None, op0=mybir.AluOpType.max)
                nc.vector.reciprocal(zr[:, 0:nbk, :], zr[:, 0:nbk, :])
                vals = ep.tile([128, EGRP, D], BF16, tag="vals")
                nc.gpsimd.tensor_tensor(
                    out=vals[:, 0:nbk, :]
                        .rearrange("p b (e h) -> p b e h", h=H),
                    in0=accs[:, :, 0:D]
                        .rearrange("p b (e h) -> p b e h", h=H),
                    in1=zr[:, 0:nbk, :]
                        .rearrange("p b (o h) -> p b o h", o=1)
                        .to_broadcast([128, nbk, HD, H]),
                    op=mybir.AluOpType.mult)
                po_sg = ep.tile([128, EGRP, D + 1], BF16, tag="posg")
                npair = (nbk + 1) // 2
                for p0 in range(0, npair, 3):
                    pn = min(3, npair - p0)
                    po = epp.tile([128, 512], F32, tag="po")
                    for pi in range(pn):
                        g = (p0 + pi) * 2
                        pt = epp.tile([128, 128], BF16, tag="pt")
                        nc.tensor.transpose(
                            out=pt[:],
                            in_=vals[:, g:g + 2, :]
                                .rearrange("p a d -> p (a d)"),
                            identity=idn_t[:])
                        vT = ep.tile([128, 128], BF16, tag="vT")
                        nc.scalar.copy(vT[:], pt[:])
                        nc.tensor.matmul(
                            out=po[:, pi * 130:pi * 130 + 130],
                            lhsT=vT[:], rhs=wot_t[:, 0:130],
                            start=True, stop=True)
                    nc.scalar.copy(
                        po_sg[:, p0 * 2:p0 * 2 + pn * 2, :]
                            .rearrange("p a d -> p (a d)"),
                        po[:, 0:pn * 130])
                nmu = po_sg[:, :, D:D + 1]      # -mean via the Wo mu column
                xpb_g = ep.tile([128, EGRP, D], BF16, tag="xpbg")
                nc.scalar.dma_start(
                    out=xpb_g[:, 0:nbk, :],
                    in_=xpb[:].rearrange("(p a) d -> p a d", p=128)
                        [:, b0:b0 + nbk, :])
                ct_g = ep.tile([128, EGRP, D], BF16, tag="ctg")
                cts = ct_g[:, 0:nbk, :]
                nc.gpsimd.tensor_tensor(
                    out=cts, in0=po_sg[:, 0:nbk, 0:D],
                    in1=nmu[:, 0:nbk, :]
                        .to_broadcast([128, nbk, D]),
                    op=mybir.AluOpType.add)
                nc.gpsimd.tensor_tensor(
                    out=cts, in0=cts,
                    in1=boc_t[:].rearrange("p (o d) -> p o d", o=1)
                        .to_broadcast([128, nbk, D]),
                    op=mybir.AluOpType.add)
                sq = ep.tile([128, EGRP, D], F32, tag="sq")
                nc.scalar.square(sq[:, 0:nbk, :], cts)
                vv_t = ep.tile([128, EGRP], F32, tag="vv")
                vv = vv_t[:, 0:nbk]
                nc.vector.tensor_reduce(
                    out=vv, in_=sq[:, 0:nbk, :],
                    axis=mybir.AxisListType.X, op=mybir.AluOpType.add)
                nc.vector.tensor_scalar(
                    out=vv, in0=vv, scalar1=1.0 / D, scalar2=LN_EPS,
                    op0=mybir.AluOpType.mult, op1=mybir.AluOpType.add)
                # rstd = var^-0.5 via exp(-0.5*ln(var)): Ln/Exp/Copy/Square
                # share one activation table set (no ATL thrash, unlike Sqrt)
                lnv = ep.tile([128, EGRP], F32, tag="lnv")
                nc.scalar.activation(
                    out=lnv[:, 0:nbk], in_=vv,
                    func=mybir.ActivationFunctionType.Ln)
                rstd = ep.tile([128, EGRP], F32, tag="rstd")
                nc.scalar.activation(
                    out=rstd[:, 0:nbk], in_=lnv[:, 0:nbk],
                    func=mybir.ActivationFunctionType.Exp, scale=-0.5)
                ot = ep.tile([128, EGRP, D], BF16, tag="ot")
                nc.gpsimd.tensor_tensor(
                    out=ot[:, 0:nbk, :], in0=cts,
                    in1=rstd[:, 0:nbk].rearrange("p (b o) -> p b o", o=1)
                        .to_broadcast([128, nbk, D]),
                    op=mybir.AluOpType.mult)
                nc.vector.tensor_tensor(
                    out=ot[:, 0:nbk, :], in0=ot[:, 0:nbk, :],
                    in1=gam_t[:].rearrange("p (o d) -> p o d", o=1)
                        .to_broadcast([128, nbk, D]),
                    op=mybir.AluOpType.mult)
                nc.vector.tensor_tensor(
                    out=ot[:, 0:nbk, :], in0=ot[:, 0:nbk, :],
                    in1=xpb_g[:, 0:nbk, :], op=mybir.AluOpType.add)
                nc.sync.dma_start(
                    out=out[:].rearrange("(p a) d -> p a d", p=128)
                        [:, b0:b0 + nbk, :],
                    in_=ot[:, 0:nbk, :])

            # ---- phase 2
            segs_by_chunk = [[] for _ in range(CHN)]
            for sg in segs:
                segs_by_chunk[sg[0]].append(sg)

            def windows(csegs, cap=256):
                """Split segments into <=cap-tile windows."""
                wins, cur, tn = [], [], 0
                for sg in csegs:
                    if cur and tn + sg[1] * sg[3] > cap:
                        wins.append(cur)
                        cur, tn = [], 0
                    cur.append(sg)
                    tn += sg[1] * sg[3]
                if cur:
                    wins.append(cur)
                return wins

            # chunks 0..2 with interleaved next-chunk table build
            for c in range(CHN - 1):
                csegs = segs_by_chunk[c]
                nseg = len(csegs)
                si = 0
                for win in windows(csegs):
                    t0 = win[0][4]
                    tn = win[-1][4] + win[-1][1] * win[-1][3] - t0
                    load_stream(t0, tn)
                    for cwin in windows(win, CWIN):
                        si += len(cwin)
                        emit_kv_slabs(int(slab_cum[c + 1])
                                      + (CH_SLABS[c + 1] * si) // nseg)
                        if si >= nseg - 1 and si - len(cwin) < nseg - 1:
                            q_gather(c + 1)
                        do_cwindow(c, cwin)

            # phase-1 pools done; free PSUM banks for the epilogue
            p1p_cm.__exit__(None, None, None)
            p1_cm.__exit__(None, None, None)
            ep_cm = tc.tile_pool(name="ep", bufs=1)
            ep = ep_cm.__enter__()
            epp_cm = tc.tile_pool(name="epps", bufs=2, space="PSUM")
            epp = epp_cm.__enter__()

            # chunk 3: combine folded into psum; epilogue per 14-block group
            cbt = []
            for ci in range(CHN - 1):
                cxi = mp.tile([128, NB // 16], I16, tag=f"cbt{ci}")
                nc.scalar.dma_start(
                    out=cxi[:],
                    in_=cbixp[:, ci * (NB // 16):(ci + 1) * (NB // 16)])
                cbt.append(cxi)
            next_grp = 0
            for win in windows(segs_by_chunk[CHN - 1]):
                t0 = win[0][4]
                tn = win[-1][4] + win[-1][1] * win[-1][3] - t0
                load_stream(t0, tn)
                for cwin in windows(win, CWIN):
                    do_cwindow(CHN - 1, cwin, cbt=cbt)
                    done_b = cwin[-1][2] + cwin[-1][3]
                    while next_grp + EGRP <= done_b:
                        epilogue_group(ep, epp, next_grp, EGRP)
                        next_grp += EGRP
            while next_grp < NBLK:
                nbk = min(EGRP, NBLK - next_grp)
                epilogue_group(ep, epp, next_grp, nbk)
                next_grp += nbk

            ep_cm.__exit__(None, None, None)
            epp_cm.__exit__(None, None, None)
            bp_cm.__exit__(None, None, None)
            wp_cm.__exit__(None, None, None)
            mp_cm.__exit__(None, None, None)
            gp_cm.__exit__(None, None, None)
    return nc


def kernel(x, edge_index, edge_weights, Wq, bq, Wk, bk, Wv, bv, Wo, bo,
           gamma, beta):
    x = np.asarray(x, np.float32)
    edge_weights = np.asarray(edge_weights, np.float32)
    origins = np.asarray(edge_index[0], np.int64)
    dests = np.asarray(edge_index[1], np.int64)

    struct, per_core, deg = _build_structure(origins, dests)
    nc = _build_graph(struct)
    nc.finalize()

    perm_t = _slab_perm(NT)
    xpad = np.zeros((NT, D), np.float32)
    xpad[:N] = x
    xT = np.empty((D + 1, NT), np.float32)
    xT[:D] = xpad[perm_t].T
    xT[D] = 1.0
    xT = xT.astype(BF16_NP)

    vperm = (np.arange(H)[None, :] * HD + np.arange(HD)[:, None]).ravel()
    wkv = np.zeros((D + 1, 2 * D), np.float32)
    wkv[:D, :D] = np.asarray(Wk, np.float32).T
    wkv[:D, D:] = np.asarray(Wv, np.float32).T[:, vperm]
    wkv[D, :D] = np.asarray(bk, np.float32)
    wkv[D, D:] = np.asarray(bv, np.float32)[vperm]
    wq = np.zeros((D + 1, D), np.float32)
    wq[:D] = np.asarray(Wq, np.float32).T
    wq[D] = np.asarray(bq, np.float32)
    wot1 = np.ascontiguousarray(np.asarray(Wo, np.float32).T[vperm, :])
    wot = np.zeros((2 * D, 2 * D + 2), np.float32)  # blkdiag + -mean columns
    wot[:D, :D] = wot1
    wot[:D, D] = -wot1.mean(axis=1)
    wot[D:, D + 1:2 * D + 1] = wot1
    wot[D:, 2 * D + 1] = -wot1.mean(axis=1)
    bo = np.asarray(bo, np.float32)
    boc = np.tile((bo - bo.mean())[None, :], (128, 1))
    gam_t = np.tile(np.asarray(gamma, np.float32)[None, :], (128, 1))
    idn = np.eye(128, dtype=np.float32)
    beta = np.asarray(beta, np.float32)
    perm_q = _slab_perm(NB)

    in_maps = []
    outs_meta = []
    for ci in range(NCORES):
        data = _per_core_arrays(struct, per_core[ci], deg[ci], edge_weights)
        xo = np.zeros((NB, D), np.float32)
        xo[:NOWN] = x[ci * NOWN:(ci + 1) * NOWN]
        xq_c = np.empty((D + 1, NB), np.float32)
        xq_c[:D] = xo[perm_q].T
        xq_c[D] = 1.0
        order3 = data["order3"]
        xpb_c = (xo[order3] + beta[None, :]).reshape(NBLK, 128, D) \
            .transpose(1, 0, 2).reshape(NB, D)
        in_maps.append({
            "xT": xT, "xq": xq_c.astype(BF16_NP),
            "wkv": wkv.astype(BF16_NP), "wq": wq.astype(BF16_NP),
            "wot": wot.astype(BF16_NP), "boc": boc.astype(BF16_NP),
            "gam": gam_t.astype(BF16_NP), "idnp": idn.astype(BF16_NP),
            "xpb": xpb_c.astype(BF16_NP), "npadp": data["npad"],
            "kvx": data["kvx"], "ewp": data["ew"],
            "qix": data["qix"], "cbix": data["cbix"],
        })
        outs_meta.append(order3)

    global LAST_SIM_NS
    if SIMULATE_COST:
        from concourse import bass_interp
        sim = bass_interp.CoreSim(nc, no_exec=True, publish_trace=False)
        sim.event_loop()
        LAST_SIM_NS = int(sim.time)

    res = run_bass_kernel_spmd(nc, in_maps, core_ids=list(range(NCORES)),
                               trace=TRACE)
    global LAST_RESULT
    LAST_RESULT = res
    outs = []
    for ci in range(NCORES):
        o = np.asarray(res.results[ci]["out"]).astype(np.float32)
        o = o.reshape(128, NBLK, D).transpose(1, 0, 2).reshape(NB, D)
        inv = np.empty(NB, np.int64)
        inv[outs_meta[ci]] = np.arange(NB)
        outs.append(o[inv[:NOWN]])
    return np.concatenate(outs, axis=0)


TRACE = False
SIMULATE_COST = False
LAST_RESULT = None
LAST_SIM_NS = None
